# revision 45
# baseline (speedup 1.0000x reference)
"""Trainium2 Bass kernel for one neural-CA (NCA) update step.

Model (per batch element, all f32):
  pre_life  = living_mask(x)                        # 3x3 circular max/avg pools on alpha=x[:,3]
  y         = depthwise 3x3 circular conv of x with 4 filters  -> [C*4, H, W]
  h         = leaky_relu(W1 @ y + b1, 0.01)         # per-pixel MLP, HID=128
  dx        = W2 @ h + b2
  xnew      = x + dx * (rand_mask <= 0.5)
  post_life = living_mask(xnew)
  out       = xnew * (pre_life & post_life)

Strategy (8 NeuronCores, pure data parallel over batch 32 -> 4 per core):
  * Fold conv+W1 into effective weights Weff[o, c, d, dj] (host precompute),
    split hi/lo in bf16: W x ~ Wh xh + Wl xh + Wh xl.
  * Both 3x3 shifts (d row, dj col) are BAKED into replica stacks: per
    quarter (32 image rows) three SBUF stacks hold the 9 (dj,d) shift
    combos x 16 channels:
      T1 [128, 4096] = xh for combos 0-7   (partition p = dj*48 + d*16 + c)
      T2 [128, 4096] = xl for combos 0-7
      T3 [49, 4096]  = [xh c8; xh c8; xl c8; ones]
    so conv+MLP1 for a 512-px chunk is 4 back-to-back matmuls (K=128,128,
    128,49) with contiguous N=512 rhs slices; b1 rides the ones row.
  * Weight-stationary: cfg-outer loop over the 8 chunks of a quarter ->
    weights swap 4x per 32 matmuls instead of per-matmul; PSUM = 4 pair
    tiles [128, 1024] (2 banks each, all 8 banks).
  * lrelu evac per pair tile, alternating ScalarE (HW Lrelu) / VectorE
    (mult+max decomp) to balance engines.
  * MLP2 (K=128, M=16->32 zero-padded, fp32) col-tiled 4x via
    tile_position; DVE evacuates dx (+b2) into a per-batch c-major tile.
  * dx transposed to H-major [H, C*W] via SBUF->SBUF DMA (no DRAM bounce);
    elementwise tail + life-mask pools run there with 128-partition tiles,
    dripped between the next batch's quarters.
"""

import os
import sys

os.environ.setdefault("JAX_PLATFORMS", "cpu")
for _p in ("/opt/trn_rl_repo", "/root/.axon_site/_ro/trn_rl_repo"):
    if os.path.isdir(_p) and _p not in sys.path:
        sys.path.insert(0, _p)

from contextlib import ExitStack

import numpy as np

import concourse.bass as bass
import concourse.tile as tile
from concourse import bacc, mybir
from concourse._compat import with_exitstack
from concourse.bass_utils import run_bass_kernel_spmd

# ----------------------------------------------------------------------------
# problem constants (hardcoded per spec nn_CAModel_2121713844629)
B, C, H, W = 32, 16, 128, 128
NF, R, K = 4, 1, 3
HID = 128
FIRE_RATE = 0.5
NEG_SLOPE = 0.01
N_CORES = 8
B_LOC = B // N_CORES          # 4 batches per core
CW = C * W                    # 2048
SW = W + 2                    # 130 padded row width (cols -1..128)
RT = H * W                    # 16384, per-channel replica plane size
QROWS = 16                    # image rows per step ("octant")
NQ = H // QROWS               # 8 octants per batch
QF = QROWS * W                # 2048 stack free size
CPQ = 4                       # 512-px chunks per octant
CHUNK = 512
NPAIR = 2                     # psum pair tiles per octant (2 chunks each)

LRELU_MODE = os.environ.get("CA_LRELU", "act")   # "act" (HW Lrelu) / "decomp"

F32 = mybir.dt.float32
BF16 = mybir.dt.bfloat16

# combo order: k = dj*3 + d ; combos 0-7 in T1/T2, combo 8 = (dj=2,d=2) in T3
COMBOS = [(dj, d) for dj in range(3) for d in range(3)]


def _avg_threshold():
    """Smallest f32 s with (np.float32(s)/9 < 0.2) False, as the strict-< bound."""
    lo = np.float32(1.7)
    hi = np.float32(1.9)
    for _ in range(80):
        mid = np.float32((lo.astype(np.float64) + hi.astype(np.float64)) / 2)
        if mid / np.float32(9.0) < np.float32(0.2):
            lo = mid
        else:
            hi = mid
    return float(hi)


AVG_LT = _avg_threshold()


# ----------------------------------------------------------------------------
@with_exitstack
def _build_kernel(ctx: ExitStack, tc: "tile.TileContext",
                  xrep1_in, xrep2_in, xrep3_in, xew_in, m_in,
                  wa1_in, wa2_in, wa4_in, w2_in, b2_in, out_dram, scr_drams):
    nc = tc.nc
    consts = ctx.enter_context(tc.tile_pool(name="consts", bufs=1))
    st1 = ctx.enter_context(tc.tile_pool(name="st1", bufs=2))
    st2 = ctx.enter_context(tc.tile_pool(name="st2", bufs=2))
    st3 = ctx.enter_context(tc.tile_pool(name="st3", bufs=2))
    hpool = ctx.enter_context(tc.tile_pool(name="hpool", bufs=6))
    dxcpool = ctx.enter_context(tc.tile_pool(name="dxc", bufs=2))
    ewpool = ctx.enter_context(tc.tile_pool(name="ewpool", bufs=2))
    small = ctx.enter_context(tc.tile_pool(name="small", bufs=1))
    psum = ctx.enter_context(tc.tile_pool(name="psum", bufs=4, space="PSUM"))

    # --- constants (weights on the scalar queue so the first stack loads
    # lead the sync queue; m_all last — phase B only) ------------------------
    wa1_t = consts.tile([128, HID], BF16)
    wa2_t = consts.tile([128, HID], BF16)
    wa4_t = consts.tile([113, HID], BF16)
    w2_t = consts.tile([HID, 32], F32)
    b2_t = consts.tile([HID, 1], F32)
    m_all = consts.tile([H, B_LOC * W], F32)

    def load_consts():
        nc.scalar.dma_start(wa1_t[:], wa1_in[:])
        nc.scalar.dma_start(wa2_t[:], wa2_in[:])
        nc.scalar.dma_start(wa4_t[:], wa4_in[:])
        nc.scalar.dma_start(w2_t[:], w2_in[:])
        nc.scalar.dma_start(b2_t[:], b2_in[:])

    state = {}

    def load_stacks(b, q):
        """Issue stack loads for (batch b, quarter q) — host-baked replicas,
        one contiguous-per-partition DMA per stack tile."""
        t1 = st1.tile([128, QF], BF16, name=f"t1_{b}_{q}", tag="t1")
        t2 = st2.tile([128, QF], BF16, name=f"t2_{b}_{q}", tag="t2")
        t3 = st3.tile([49, QF], BF16, name=f"t3_{b}_{q}", tag="t3")
        for src_dram, dst, npart, eng in ((xrep1_in, t1, 128, nc.sync),
                                          (xrep2_in, t2, 128, nc.scalar),
                                          (xrep3_in, t3, 49, nc.sync)):
            srcap = bass.AP(
                tensor=src_dram.tensor,
                offset=src_dram.offset + (b * npart) * RT + q * QF,
                ap=[[RT, npart], [1, QF]])
            eng.dma_start(dst[0:npart, :], srcap)
        state["stacks", b, q] = (t1, t2, t3)

    def mlp1_part(b, q, drip):
        """MLP1 MMs + lrelu evacs for octant (b, q); returns h_sb pair."""
        t1, t2, t3 = state.pop(("stacks", b, q))
        hps = [psum.tile([HID, 2 * CHUNK], F32, name=f"hps{b}_{q}_{p}",
                         tag="hps") for p in range(NPAIR)]

        def rhs(st, cl, kp):
            return bass.AP(tensor=st.tensor, offset=st.offset + cl * CHUNK,
                           ap=[[QF, kp], [1, CHUNK]])

        cfgs = ((wa1_t, t1, 128), (wa1_t, t2, 128),
                (wa2_t, t1, 128), (wa4_t, t3, 49))

        def evac(p):
            """lrelu evac of pair p — ScalarE only: the DVE queue belongs to
            the phase-B tail, whose long dependency chains would otherwise
            block evacs (strict FIFO) and stall MLP2 + the PSUM rotation."""
            hs = hpool.tile([HID, 2 * CHUNK], F32, name=f"h{b}_{q}_{p}",
                            tag="h_sb")
            if LRELU_MODE == "act":
                nc.scalar.activation(hs[:], hps[p][:],
                                     mybir.ActivationFunctionType.Lrelu,
                                     scale=1.0, alpha=NEG_SLOPE)
            else:
                tt = small.tile([HID, 2 * CHUNK], F32, name=f"lt{b}_{q}_{p}",
                                tag="ltmp", bufs=2)
                nc.vector.tensor_scalar(tt[:], hps[p][:], NEG_SLOPE, None,
                                        op0=mybir.AluOpType.mult)
                nc.vector.tensor_tensor(hs[:], hps[p][:], tt[:],
                                        op=mybir.AluOpType.max)
            return hs

        h_sb = [None] * NPAIR
        for ci, (wt, st, kp) in enumerate(cfgs):
            for cl in range(CPQ):
                dst = hps[cl // 2][:, (cl % 2) * CHUNK:
                                   (cl % 2 + 1) * CHUNK]
                nc.tensor.matmul(dst, wt[0:kp, :], rhs(st, cl, kp),
                                 start=(ci == 0), stop=(ci == 3))
        for p in range(NPAIR):
            h_sb[p] = evac(p)
        drip()
        return h_sb

    def mlp2_part(b, q, h_sb, drip):
        """MLP2 + dx evac + dumps for octant (b, q) — issued one octant
        late so the in-order PE queue never waits on the lrelu evacs."""
        # MLP2: one group of 4 chunks, col-tiled fp32
        dxp = psum.tile([HID, CHUNK], F32, name=f"dxp{b}_{q}", tag="hps")
        for j in range(4):
            nc.tensor.matmul(
                dxp[32 * j:32 * j + 32, :],
                w2_t[:, 0:32],
                h_sb[j // 2][:, (j % 2) * CHUNK:(j % 2 + 1) * CHUNK],
                start=True, stop=True, tile_position=(0, 32 * j))
        # dx evac + b2 into c-major staging (ScalarE), then dump H-major
        # into the DRAM scratch (the partition<->free transpose can only
        # happen through a DRAM-side free-form AP)
        dxq = dxcpool.tile([HID, CHUNK], F32, name=f"dxq{b}_{q}",
                           tag="dxq")
        nc.scalar.activation(dxq[:], dxp[:],
                             mybir.ActivationFunctionType.Identity,
                             bias=b2_t[:], scale=1.0)
        scr = scr_drams[b]
        for j in range(4):
            eng = nc.gpsimd if j < 2 else nc.sync
            srcap = bass.AP(
                tensor=dxq.tensor,
                offset=dxq.offset + (32 * j) * CHUNK,
                ap=[[CHUNK, C], [W, 4], [1, W]])
            dstap = bass.AP(
                tensor=scr.tensor,
                offset=scr.offset + (16 * q + 4 * j) * CW,
                ap=[[W, C], [CW, 4], [1, W]])
            eng.dma_start(dstap, srcap)
        drip()

    def transpose_dx(b):
        """dxc [32j+c, g*512+r*128+w] -> dx_ew [16g+4j+r, c*128+w]."""
        dx_ew = ewpool.tile([H, CW], F32, name=f"dxew{b}", tag="dx_ew")
        nc.scalar.dma_start(dx_ew[:],
                            scr_drams[b].rearrange("h c w -> h (c w)"))
        state["dx_ew", b] = dx_ew

    def load_xew(b):
        x_ew = ewpool.tile([H, CW], F32, name=f"xew{b}", tag="x_ew", bufs=3)
        nc.gpsimd.dma_start(x_ew[:], xew_in[b])
        state["x_ew", b] = x_ew

    def phase_B_bundles(b):
        """elementwise tail + life masks + store for batch b (thunk list)."""
        x_ew = state.pop(("x_ew", b))
        st = {}

        def bcast(t128):
            return bass.AP(tensor=t128.tensor, offset=t128.offset,
                           ap=[[t128.ap[0][0], H], [0, C], [1, W]])

        def bn_ew():
            dx_ew = state.pop(("dx_ew", b))
            m_b = bass.AP(tensor=m_all.tensor, offset=m_all.offset + b * W,
                          ap=[[m_all.ap[0][0], H], [0, C], [1, W]])
            nc.vector.tensor_tensor(dx_ew[:], dx_ew[:], m_b,
                                    op=mybir.AluOpType.mult)
            xnew = ewpool.tile([H, CW], F32, name=f"xnew{b}", tag="xnew")
            nc.vector.tensor_tensor(xnew[:], x_ew[:], dx_ew[:],
                                    op=mybir.AluOpType.add)
            st["xnew"] = xnew

        def living(src_getter, which):
            def fn():
                src_ew = src_getter()
                ap_pad = small.tile([H, SW], F32, name=f"ap{which}{b}",
                                    tag=f"ap{which}")
                alpha = src_ew[:, 3 * W:4 * W]
                nc.vector.tensor_copy(ap_pad[:, 1:1 + W], alpha)
                nc.vector.tensor_copy(ap_pad[:, 0:1],
                                      src_ew[:, 4 * W - 1:4 * W])
                nc.vector.tensor_copy(ap_pad[:, 1 + W:2 + W],
                                      src_ew[:, 3 * W:3 * W + 1])
                hh = small.tile([H, 2 * W], F32, name=f"hh{which}{b}",
                                tag=f"hh{which}")
                hm = hh[:, 0:W]
                hs = hh[:, W:2 * W]
                nc.vector.tensor_tensor(hm, ap_pad[:, 0:W], ap_pad[:, 1:1 + W],
                                        op=mybir.AluOpType.max)
                nc.vector.tensor_tensor(hm, hm, ap_pad[:, 2:2 + W],
                                        op=mybir.AluOpType.max)
                nc.vector.tensor_tensor(hs, ap_pad[:, 0:W], ap_pad[:, 1:1 + W],
                                        op=mybir.AluOpType.add)
                nc.vector.tensor_tensor(hs, hs, ap_pad[:, 2:2 + W],
                                        op=mybir.AluOpType.add)
                up = small.tile([H, 2 * W], F32, name=f"up{which}{b}",
                                tag=f"up{which}")
                dn = small.tile([H, 2 * W], F32, name=f"dn{which}{b}",
                                tag=f"dn{which}")
                nc.gpsimd.dma_start(up[0:H - 1, :], hh[1:H, :])
                nc.gpsimd.dma_start(up[H - 1:H, :], hh[0:1, :])
                nc.gpsimd.dma_start(dn[1:H, :], hh[0:H - 1, :])
                nc.gpsimd.dma_start(dn[0:1, :], hh[H - 1:H, :])
                st[f"hh{which}"] = (hh, up, dn)
            return fn

        def living_v(which):
            def fn():
                hh, up, dn = st.pop(f"hh{which}")
                vm = small.tile([H, W], F32, name=f"vm{which}{b}",
                                tag=f"vm{which}")
                vs = small.tile([H, W], F32, name=f"vs{which}{b}",
                                tag=f"vs{which}")
                for (t_out, o0, op) in ((vm, 0, mybir.AluOpType.max),
                                        (vs, W, mybir.AluOpType.add)):
                    nc.vector.tensor_tensor(t_out[:], hh[:, o0:o0 + W],
                                            up[:, o0:o0 + W], op=op)
                    nc.vector.tensor_tensor(t_out[:], t_out[:],
                                            dn[:, o0:o0 + W], op=op)
                alive = small.tile([H, W], F32, name=f"al{which}{b}",
                                   tag=f"al{which}")
                nc.vector.tensor_scalar(alive[:], vm[:], 0.1, None,
                                        op0=mybir.AluOpType.is_gt)
                avgok = small.tile([H, W], F32, name=f"ag{which}{b}",
                                   tag=f"ag{which}")
                nc.vector.tensor_scalar(avgok[:], vs[:], AVG_LT, None,
                                        op0=mybir.AluOpType.is_lt)
                lif = small.tile([H, W], F32, name=f"lf{which}{b}",
                                 tag=f"lf{which}")
                nc.vector.tensor_tensor(lif[:], alive[:], avgok[:],
                                        op=mybir.AluOpType.mult)
                st[f"life{which}"] = lif
            return fn

        def bn_final():
            xnew = st["xnew"]
            life = small.tile([H, W], F32, name=f"life{b}", tag="life")
            nc.vector.tensor_tensor(life[:], st["lifepre"][:],
                                    st["lifepost"][:],
                                    op=mybir.AluOpType.mult)
            nc.vector.tensor_tensor(xnew[:], xnew[:], bcast(life),
                                    op=mybir.AluOpType.mult)
            nc.sync.dma_start(out_dram[b], xnew[:])

        pre = [
            living(lambda: x_ew, "pre"),
            living_v("pre"),
        ]
        post = [
            bn_ew,
            living(lambda: st["xnew"], "post"),
            living_v("post"),
            bn_final,
        ]
        return pre, post

    # --- pipeline: (b, q) steps, dripping phase-B thunks between MM blocks:
    # pre-life of batch b runs during b's own quarters; the tail (xnew,
    # post-life, store) runs during b+1's quarters.
    pending = []

    def drip():
        if pending:
            pending.pop(0)()

    NSTEP = B_LOC * NQ
    bundles = {}
    load_stacks(0, 0)
    load_consts()
    load_stacks(0, 1)
    prev = None
    for step in range(NSTEP):
        b, q = divmod(step, NQ)
        if step == 1:
            # deferred past the startup burst: only needed by phase B
            load_xew(0)
            nc.gpsimd.dma_start(m_all[:], m_in[:])
        if step + 2 < NSTEP:
            nb, nq = divmod(step + 2, NQ)
            load_stacks(nb, nq)
            if nq == 0:
                load_xew(nb)
        if q == 1:
            pre, post = phase_B_bundles(b)
            bundles[b] = post
            pending.extend(pre)
        h_sb = mlp1_part(b, q, drip)
        if prev is not None:
            pb, pq, ph = prev
            mlp2_part(pb, pq, ph, drip)
            if pq == NQ - 1:
                transpose_dx(pb)
                while pending:
                    drip()
                pending = bundles.pop(pb)
        prev = (b, q, h_sb)
    pb, pq, ph = prev
    mlp2_part(pb, pq, ph, drip)
    transpose_dx(pb)
    while pending:
        drip()
    pending = bundles.pop(pb)
    while pending:
        drip()


# ----------------------------------------------------------------------------
_PROGRAM_CACHE = {}


def _get_program():
    key = (LRELU_MODE,)
    if key in _PROGRAM_CACHE:
        return _PROGRAM_CACHE[key]
    nc = bacc.Bacc("TRN2", target_bir_lowering=False, debug=False,
                   num_devices=N_CORES)
    xrep1_in = nc.dram_tensor("xrep1_in", [B_LOC, 128, RT], BF16,
                              kind="ExternalInput").ap()
    xrep2_in = nc.dram_tensor("xrep2_in", [B_LOC, 128, RT], BF16,
                              kind="ExternalInput").ap()
    xrep3_in = nc.dram_tensor("xrep3_in", [B_LOC, 49, RT], BF16,
                              kind="ExternalInput").ap()
    xew_in = nc.dram_tensor("xew_in", [B_LOC, H, CW], F32,
                            kind="ExternalInput").ap()
    m_in = nc.dram_tensor("m_in", [H, B_LOC * W], F32,
                          kind="ExternalInput").ap()
    wa1_in = nc.dram_tensor("wa1_in", [128, HID], BF16,
                            kind="ExternalInput").ap()
    wa2_in = nc.dram_tensor("wa2_in", [128, HID], BF16,
                            kind="ExternalInput").ap()
    wa4_in = nc.dram_tensor("wa4_in", [113, HID], BF16,
                            kind="ExternalInput").ap()
    w2_in = nc.dram_tensor("w2_in", [HID, 32], F32, kind="ExternalInput").ap()
    b2_in = nc.dram_tensor("b2_in", [HID, 1], F32, kind="ExternalInput").ap()
    out_dram = nc.dram_tensor("out", [B_LOC, H, CW], F32,
                              kind="ExternalOutput").ap()
    scr_drams = [nc.dram_tensor(f"dxscr{b}", [H, C, W], F32).ap()
                 for b in range(B_LOC)]
    with tile.TileContext(nc) as tc:
        _build_kernel(tc, xrep1_in, xrep2_in, xrep3_in, xew_in, m_in,
                      wa1_in, wa2_in, wa4_in, w2_in, b2_in, out_dram,
                      scr_drams)
    nc.compile()
    _PROGRAM_CACHE[key] = nc
    return nc


def _host_weights(filters, W1, b1, W2, b2):
    import ml_dtypes
    filters = np.asarray(filters, np.float32)
    W1 = np.asarray(W1, np.float32)
    W2 = np.asarray(W2, np.float32)
    b1 = np.asarray(b1, np.float32)
    b2 = np.asarray(b2, np.float32)
    # Weff[o, c, d, dj] = sum_f W1[o, c*NF+f] * filters[f, d, dj]
    w1r = W1.reshape(HID, C, NF)
    weff = np.einsum("ocf,fij->ocij", w1r, filters)       # [o, c, d, dj]
    wah = weff.astype(ml_dtypes.bfloat16)
    wal = (weff - wah.astype(np.float32)).astype(ml_dtypes.bfloat16)
    wa1 = np.zeros((128, HID), ml_dtypes.bfloat16)
    wa2 = np.zeros((128, HID), ml_dtypes.bfloat16)
    for k, (dj, d) in enumerate(COMBOS[:8]):
        wa1[k * C:(k + 1) * C, :] = wah[:, :, d, dj].T
        wa2[k * C:(k + 1) * C, :] = wal[:, :, d, dj].T
    wa4 = np.zeros((113, HID), ml_dtypes.bfloat16)
    wa4[0:C, :] = wah[:, :, 2, 2].T
    wa4[C:2 * C, :] = wal[:, :, 2, 2].T
    wa4[2 * C:3 * C, :] = wah[:, :, 2, 2].T
    wa4[48, :] = b1.astype(ml_dtypes.bfloat16)
    wa4[64:113, :] = wa4[0:49, :]
    w2p = np.zeros((HID, 32), np.float32)
    w2p[:, :C] = W2.T
    b2v = np.zeros((HID, 1), np.float32)
    for j in range(4):
        b2v[32 * j:32 * j + C, 0] = b2
    return wa1, wa2, wa4, w2p, b2v


def _prepare_in_maps(x, rand_mask, filters, W1, b1, W2, b2):
    import ml_dtypes
    x = np.asarray(x, np.float32)
    # replica planes: xp rows/cols -1..128 (wrap); combo (dj,d) plane =
    # xp[:, :, d:d+128, dj:dj+128] == x[c, i+d-1, j+dj-1]
    xp = np.pad(x, ((0, 0), (0, 0), (1, 1), (1, 1)), mode="wrap")
    xp_h = xp.astype(ml_dtypes.bfloat16)
    xp_l = (xp - xp_h.astype(np.float32)).astype(ml_dtypes.bfloat16)
    xrep1 = np.empty((B, 8, C, H, W), ml_dtypes.bfloat16)
    xrep2 = np.empty((B, 8, C, H, W), ml_dtypes.bfloat16)
    for k, (dj, d) in enumerate(COMBOS[:8]):
        xrep1[:, k] = xp_h[:, :, d:d + H, dj:dj + W]
        xrep2[:, k] = xp_l[:, :, d:d + H, dj:dj + W]
    xrep1 = xrep1.reshape(B, 128, RT)
    xrep2 = xrep2.reshape(B, 128, RT)
    xrep3 = np.empty((B, 49, H, W), ml_dtypes.bfloat16)
    xrep3[:, 0:C] = xp_h[:, :, 2:2 + H, 2:2 + W]
    xrep3[:, C:2 * C] = xrep3[:, 0:C]
    xrep3[:, 2 * C:3 * C] = xp_l[:, :, 2:2 + H, 2:2 + W]
    xrep3[:, 48] = np.ones((B, H, W), ml_dtypes.bfloat16)
    xrep3 = xrep3.reshape(B, 49, RT)
    xew = np.ascontiguousarray(x.transpose(0, 2, 1, 3).reshape(B, H, CW))
    m = (np.asarray(rand_mask, np.float32)
         <= np.float32(FIRE_RATE)).astype(np.float32)
    m = m.reshape(B, H, W).transpose(1, 0, 2)   # [H, B, W]
    wa1, wa2, wa4, w2p, b2v = _host_weights(filters, W1, b1, W2, b2)
    in_maps = []
    for core in range(N_CORES):
        sl = slice(core * B_LOC, (core + 1) * B_LOC)
        in_maps.append({
            "xrep1_in": xrep1[sl], "xrep2_in": xrep2[sl],
            "xrep3_in": xrep3[sl], "xew_in": xew[sl],
            "m_in": np.ascontiguousarray(m[:, sl, :]).reshape(H, B_LOC * W),
            "wa1_in": wa1, "wa2_in": wa2, "wa4_in": wa4,
            "w2_in": w2p, "b2_in": b2v,
        })
    return in_maps


def kernel(x, rand_mask, filters, W1, b1, W2, b2, _want_trace=False):
    in_maps = _prepare_in_maps(x, rand_mask, filters, W1, b1, W2, b2)
    nc = _get_program()
    res = run_bass_kernel_spmd(nc, in_maps, list(range(N_CORES)),
                               trace=_want_trace)
    out = np.concatenate([res.results[i]["out"] for i in range(N_CORES)],
                         axis=0)
    out = np.ascontiguousarray(
        out.reshape(B, H, C, W).transpose(0, 2, 1, 3))
    if _want_trace:
        return out, res
    return out


# revision 50
# speedup vs baseline: 1.0308x; 1.0308x over previous
"""Trainium2 Bass kernel for one neural-CA (NCA) update step.

Model (per batch element, all f32):
  pre_life  = living_mask(x)                        # 3x3 circular max/avg pools on alpha=x[:,3]
  y         = depthwise 3x3 circular conv of x with 4 filters  -> [C*4, H, W]
  h         = leaky_relu(W1 @ y + b1, 0.01)         # per-pixel MLP, HID=128
  dx        = W2 @ h + b2
  xnew      = x + dx * (rand_mask <= 0.5)
  post_life = living_mask(xnew)
  out       = xnew * (pre_life & post_life)

Strategy (8 NeuronCores, pure data parallel over batch 32 -> 4 per core):
  * Fold conv+W1 into effective weights Weff[o, c, d, dj] (host precompute),
    split hi/lo in bf16: W x ~ Wh xh + Wl xh + Wh xl.
  * Both 3x3 shifts (d row, dj col) are BAKED into replica stacks: per
    quarter (32 image rows) three SBUF stacks hold the 9 (dj,d) shift
    combos x 16 channels:
      T1 [128, 4096] = xh for combos 0-7   (partition p = dj*48 + d*16 + c)
      T2 [128, 4096] = xl for combos 0-7
      T3 [49, 4096]  = [xh c8; xh c8; xl c8; ones]
    so conv+MLP1 for a 512-px chunk is 4 back-to-back matmuls (K=128,128,
    128,49) with contiguous N=512 rhs slices; b1 rides the ones row.
  * Weight-stationary: cfg-outer loop over the 8 chunks of a quarter ->
    weights swap 4x per 32 matmuls instead of per-matmul; PSUM = 4 pair
    tiles [128, 1024] (2 banks each, all 8 banks).
  * lrelu evac per pair tile, alternating ScalarE (HW Lrelu) / VectorE
    (mult+max decomp) to balance engines.
  * MLP2 (K=128, M=16->32 zero-padded, fp32) col-tiled 4x via
    tile_position; DVE evacuates dx (+b2) into a per-batch c-major tile.
  * dx transposed to H-major [H, C*W] via SBUF->SBUF DMA (no DRAM bounce);
    elementwise tail + life-mask pools run there with 128-partition tiles,
    dripped between the next batch's quarters.
"""

import os
import sys

os.environ.setdefault("JAX_PLATFORMS", "cpu")
for _p in ("/opt/trn_rl_repo", "/root/.axon_site/_ro/trn_rl_repo"):
    if os.path.isdir(_p) and _p not in sys.path:
        sys.path.insert(0, _p)

from contextlib import ExitStack

import numpy as np

import concourse.bass as bass
import concourse.tile as tile
from concourse import bacc, mybir
from concourse._compat import with_exitstack
from concourse.bass_utils import run_bass_kernel_spmd

# ----------------------------------------------------------------------------
# problem constants (hardcoded per spec nn_CAModel_2121713844629)
B, C, H, W = 32, 16, 128, 128
NF, R, K = 4, 1, 3
HID = 128
FIRE_RATE = 0.5
NEG_SLOPE = 0.01
N_CORES = 8
B_LOC = B // N_CORES          # 4 batches per core
CW = C * W                    # 2048
SW = W + 2                    # 130 padded row width (cols -1..128)
RT = H * W                    # 16384, per-channel replica plane size
QROWS = 16                    # image rows per step ("octant")
NQ = H // QROWS               # 8 octants per batch
QF = QROWS * W                # 2048 stack free size
CPQ = 4                       # 512-px chunks per octant
CHUNK = 512
NPAIR = 2                     # psum pair tiles per octant (2 chunks each)

LRELU_MODE = os.environ.get("CA_LRELU", "act")   # "act" (HW Lrelu) / "decomp"

F32 = mybir.dt.float32
BF16 = mybir.dt.bfloat16

# combo order: k = dj*3 + d ; combos 0-7 in T1/T2, combo 8 = (dj=2,d=2) in T3
COMBOS = [(dj, d) for dj in range(3) for d in range(3)]


def _avg_threshold():
    """Smallest f32 s with (np.float32(s)/9 < 0.2) False, as the strict-< bound."""
    lo = np.float32(1.7)
    hi = np.float32(1.9)
    for _ in range(80):
        mid = np.float32((lo.astype(np.float64) + hi.astype(np.float64)) / 2)
        if mid / np.float32(9.0) < np.float32(0.2):
            lo = mid
        else:
            hi = mid
    return float(hi)


AVG_LT = _avg_threshold()


# ----------------------------------------------------------------------------
@with_exitstack
def _build_kernel(ctx: ExitStack, tc: "tile.TileContext",
                  xrep1_in, xrep2_in, xrep3_in, xew_in, m_in,
                  wa1_in, wa2_in, wa4_in, w2_in, b2_in, out_dram, scr_drams):
    nc = tc.nc
    consts = ctx.enter_context(tc.tile_pool(name="consts", bufs=1))
    st1 = ctx.enter_context(tc.tile_pool(name="st1", bufs=4))
    st2 = ctx.enter_context(tc.tile_pool(name="st2", bufs=4))
    st3 = ctx.enter_context(tc.tile_pool(name="st3", bufs=4))
    hpool = ctx.enter_context(tc.tile_pool(name="hpool", bufs=6))
    dxcpool = ctx.enter_context(tc.tile_pool(name="dxc", bufs=3))
    ewpool = ctx.enter_context(tc.tile_pool(name="ewpool", bufs=2))
    small = ctx.enter_context(tc.tile_pool(name="small", bufs=1))
    psum = ctx.enter_context(tc.tile_pool(name="psum", bufs=4, space="PSUM"))

    # --- constants (weights on the scalar queue so the first stack loads
    # lead the sync queue; m_all last — phase B only) ------------------------
    wa1_t = consts.tile([128, HID], BF16)
    wa2_t = consts.tile([128, HID], BF16)
    wa4_t = consts.tile([113, HID], BF16)
    w2_t = consts.tile([HID, 32], F32)
    b2_t = consts.tile([HID, 1], F32)
    m_all = consts.tile([H, B_LOC * W], F32)

    def load_consts():
        nc.scalar.dma_start(wa1_t[:], wa1_in[:])
        nc.scalar.dma_start(wa2_t[:], wa2_in[:])
        nc.scalar.dma_start(wa4_t[:], wa4_in[:])
        nc.scalar.dma_start(w2_t[:], w2_in[:])
        nc.scalar.dma_start(b2_t[:], b2_in[:])

    state = {}

    def load_stacks(b, q):
        """Issue stack loads for (batch b, quarter q) — host-baked replicas,
        one contiguous-per-partition DMA per stack tile."""
        t1 = st1.tile([128, QF], BF16, name=f"t1_{b}_{q}", tag="t1")
        t2 = st2.tile([128, QF], BF16, name=f"t2_{b}_{q}", tag="t2")
        t3 = st3.tile([49, QF], BF16, name=f"t3_{b}_{q}", tag="t3")
        # all stack loads on the sync queue: the scalar queue carries the
        # ACT evacuations and would delay the T2 trigger behind them
        for src_dram, dst, npart, eng in ((xrep1_in, t1, 128, nc.sync),
                                          (xrep2_in, t2, 128, nc.sync),
                                          (xrep3_in, t3, 49, nc.sync)):
            srcap = bass.AP(
                tensor=src_dram.tensor,
                offset=src_dram.offset + (b * npart) * RT + q * QF,
                ap=[[RT, npart], [1, QF]])
            eng.dma_start(dst[0:npart, :], srcap)
        state["stacks", b, q] = (t1, t2, t3)

    def mlp1_part(b, q, drip):
        """MLP1 MMs + lrelu evacs for octant (b, q); returns h_sb pair."""
        t1, t2, t3 = state.pop(("stacks", b, q))
        hps = [psum.tile([HID, 2 * CHUNK], F32, name=f"hps{b}_{q}_{p}",
                         tag="hps") for p in range(NPAIR)]

        def rhs(st, cl, kp):
            return bass.AP(tensor=st.tensor, offset=st.offset + cl * CHUNK,
                           ap=[[QF, kp], [1, CHUNK]])

        cfgs = ((wa1_t, t1, 128), (wa1_t, t2, 128),
                (wa2_t, t1, 128), (wa4_t, t3, 49))

        def evac(p):
            """lrelu evac of pair p: p0 on ScalarE, p1 on VectorE — safe to
            share DVE with phase-B thunks now that MLP2 consumption is
            deferred a full octant."""
            hs = hpool.tile([HID, 2 * CHUNK], F32, name=f"h{b}_{q}_{p}",
                            tag="h_sb")
            if LRELU_MODE == "act" and p == 0:
                nc.scalar.activation(hs[:], hps[p][:],
                                     mybir.ActivationFunctionType.Lrelu,
                                     scale=1.0, alpha=NEG_SLOPE)
            else:
                tt = small.tile([HID, 2 * CHUNK], F32, name=f"lt{b}_{q}_{p}",
                                tag="ltmp", bufs=2)
                nc.vector.tensor_scalar(tt[:], hps[p][:], NEG_SLOPE, None,
                                        op0=mybir.AluOpType.mult)
                nc.vector.tensor_tensor(hs[:], hps[p][:], tt[:],
                                        op=mybir.AluOpType.max)
            return hs

        h_sb = [None] * NPAIR
        for ci, (wt, st, kp) in enumerate(cfgs):
            for cl in range(CPQ):
                dst = hps[cl // 2][:, (cl % 2) * CHUNK:
                                   (cl % 2 + 1) * CHUNK]
                nc.tensor.matmul(dst, wt[0:kp, :], rhs(st, cl, kp),
                                 start=(ci == 0), stop=(ci == 3))
        for p in range(NPAIR):
            h_sb[p] = evac(p)
        drip()
        return h_sb

    def mlp2_part(b, q, h_sb, drip):
        """MLP2 + dx evac + dumps for octant (b, q) — issued one octant
        late so the in-order PE queue never waits on the lrelu evacs."""
        # MLP2: one group of 4 chunks, col-tiled fp32
        dxp = psum.tile([HID, CHUNK], F32, name=f"dxp{b}_{q}", tag="hps")
        for j in range(4):
            nc.tensor.matmul(
                dxp[32 * j:32 * j + 32, :],
                w2_t[:, 0:32],
                h_sb[j // 2][:, (j % 2) * CHUNK:(j % 2 + 1) * CHUNK],
                start=True, stop=True, tile_position=(0, 32 * j))
        # dx evac + b2 into c-major staging (ScalarE), then dump H-major
        # into the DRAM scratch (the partition<->free transpose can only
        # happen through a DRAM-side free-form AP)
        dxq = dxcpool.tile([HID, CHUNK], F32, name=f"dxq{b}_{q}",
                           tag="dxq")
        nc.scalar.activation(dxq[:], dxp[:],
                             mybir.ActivationFunctionType.Identity,
                             bias=b2_t[:], scale=1.0)
        scr = scr_drams[b]
        for j in range(4):
            eng = nc.gpsimd if j < 2 else nc.sync
            srcap = bass.AP(
                tensor=dxq.tensor,
                offset=dxq.offset + (32 * j) * CHUNK,
                ap=[[CHUNK, C], [W, 4], [1, W]])
            dstap = bass.AP(
                tensor=scr.tensor,
                offset=scr.offset + (16 * q + 4 * j) * CW,
                ap=[[W, C], [CW, 4], [1, W]])
            eng.dma_start(dstap, srcap)
        drip()

    def transpose_dx(b):
        """dxc [32j+c, g*512+r*128+w] -> dx_ew [16g+4j+r, c*128+w]."""
        dx_ew = ewpool.tile([H, CW], F32, name=f"dxew{b}", tag="dx_ew")
        nc.gpsimd.dma_start(dx_ew[:],
                            scr_drams[b].rearrange("h c w -> h (c w)"))
        state["dx_ew", b] = dx_ew

    def load_xew(b):
        x_ew = ewpool.tile([H, CW], F32, name=f"xew{b}", tag="x_ew", bufs=3)
        nc.gpsimd.dma_start(x_ew[:], xew_in[b])
        state["x_ew", b] = x_ew

    def phase_B_bundles(b):
        """elementwise tail + life masks + store for batch b (thunk list)."""
        x_ew = state.pop(("x_ew", b))
        st = {}

        def bcast(t128):
            return bass.AP(tensor=t128.tensor, offset=t128.offset,
                           ap=[[t128.ap[0][0], H], [0, C], [1, W]])

        def bn_ew():
            dx_ew = state.pop(("dx_ew", b))
            m_b = bass.AP(tensor=m_all.tensor, offset=m_all.offset + b * W,
                          ap=[[m_all.ap[0][0], H], [0, C], [1, W]])
            nc.vector.tensor_tensor(dx_ew[:], dx_ew[:], m_b,
                                    op=mybir.AluOpType.mult)
            xnew = ewpool.tile([H, CW], F32, name=f"xnew{b}", tag="xnew")
            nc.vector.tensor_tensor(xnew[:], x_ew[:], dx_ew[:],
                                    op=mybir.AluOpType.add)
            st["xnew"] = xnew

        def living(src_getter, which):
            def fn():
                src_ew = src_getter()
                ap_pad = small.tile([H, SW], F32, name=f"ap{which}{b}",
                                    tag=f"ap{which}")
                alpha = src_ew[:, 3 * W:4 * W]
                nc.vector.tensor_copy(ap_pad[:, 1:1 + W], alpha)
                nc.vector.tensor_copy(ap_pad[:, 0:1],
                                      src_ew[:, 4 * W - 1:4 * W])
                nc.vector.tensor_copy(ap_pad[:, 1 + W:2 + W],
                                      src_ew[:, 3 * W:3 * W + 1])
                hh = small.tile([H, 2 * W], F32, name=f"hh{which}{b}",
                                tag=f"hh{which}")
                hm = hh[:, 0:W]
                hs = hh[:, W:2 * W]
                nc.vector.tensor_tensor(hm, ap_pad[:, 0:W], ap_pad[:, 1:1 + W],
                                        op=mybir.AluOpType.max)
                nc.vector.tensor_tensor(hm, hm, ap_pad[:, 2:2 + W],
                                        op=mybir.AluOpType.max)
                nc.vector.tensor_tensor(hs, ap_pad[:, 0:W], ap_pad[:, 1:1 + W],
                                        op=mybir.AluOpType.add)
                nc.vector.tensor_tensor(hs, hs, ap_pad[:, 2:2 + W],
                                        op=mybir.AluOpType.add)
                up = small.tile([H, 2 * W], F32, name=f"up{which}{b}",
                                tag=f"up{which}")
                dn = small.tile([H, 2 * W], F32, name=f"dn{which}{b}",
                                tag=f"dn{which}")
                nc.gpsimd.dma_start(up[0:H - 1, :], hh[1:H, :])
                nc.gpsimd.dma_start(up[H - 1:H, :], hh[0:1, :])
                nc.gpsimd.dma_start(dn[1:H, :], hh[0:H - 1, :])
                nc.gpsimd.dma_start(dn[0:1, :], hh[H - 1:H, :])
                st[f"hh{which}"] = (hh, up, dn)
            return fn

        def living_v(which):
            def fn():
                hh, up, dn = st.pop(f"hh{which}")
                vm = small.tile([H, W], F32, name=f"vm{which}{b}",
                                tag=f"vm{which}")
                vs = small.tile([H, W], F32, name=f"vs{which}{b}",
                                tag=f"vs{which}")
                for (t_out, o0, op) in ((vm, 0, mybir.AluOpType.max),
                                        (vs, W, mybir.AluOpType.add)):
                    nc.vector.tensor_tensor(t_out[:], hh[:, o0:o0 + W],
                                            up[:, o0:o0 + W], op=op)
                    nc.vector.tensor_tensor(t_out[:], t_out[:],
                                            dn[:, o0:o0 + W], op=op)
                alive = small.tile([H, W], F32, name=f"al{which}{b}",
                                   tag=f"al{which}")
                nc.vector.tensor_scalar(alive[:], vm[:], 0.1, None,
                                        op0=mybir.AluOpType.is_gt)
                avgok = small.tile([H, W], F32, name=f"ag{which}{b}",
                                   tag=f"ag{which}")
                nc.vector.tensor_scalar(avgok[:], vs[:], AVG_LT, None,
                                        op0=mybir.AluOpType.is_lt)
                lif = small.tile([H, W], F32, name=f"lf{which}{b}",
                                 tag=f"lf{which}")
                nc.vector.tensor_tensor(lif[:], alive[:], avgok[:],
                                        op=mybir.AluOpType.mult)
                st[f"life{which}"] = lif
            return fn

        def bn_final():
            xnew = st["xnew"]
            life = small.tile([H, W], F32, name=f"life{b}", tag="life")
            nc.vector.tensor_tensor(life[:], st["lifepre"][:],
                                    st["lifepost"][:],
                                    op=mybir.AluOpType.mult)
            nc.vector.tensor_tensor(xnew[:], xnew[:], bcast(life),
                                    op=mybir.AluOpType.mult)
            nc.sync.dma_start(out_dram[b], xnew[:])

        pre = [
            living(lambda: x_ew, "pre"),
            living_v("pre"),
        ]
        post = [
            bn_ew,
            living(lambda: st["xnew"], "post"),
            living_v("post"),
            bn_final,
        ]
        return pre, post

    # --- pipeline: (b, q) steps, dripping phase-B thunks between MM blocks:
    # pre-life of batch b runs during b's own quarters; the tail (xnew,
    # post-life, store) runs during b+1's quarters.
    pending = []

    def drip():
        if pending:
            pending.pop(0)()

    NSTEP = B_LOC * NQ
    PREFETCH = 4
    bundles = {}
    load_stacks(0, 0)
    load_consts()
    for pq0 in range(1, PREFETCH):
        load_stacks(0, pq0)
    prev = None
    for step in range(NSTEP):
        b, q = divmod(step, NQ)
        if step == 1:
            # deferred past the startup burst: only needed by phase B
            load_xew(0)
            nc.gpsimd.dma_start(m_all[:], m_in[:])
        if q == 1:
            pre, post = phase_B_bundles(b)
            bundles[b] = post
            pending.extend(pre)
        h_sb = mlp1_part(b, q, drip)
        if prev is not None:
            pb, pq, ph = prev
            mlp2_part(pb, pq, ph, drip)
            if pq == NQ - 1:
                transpose_dx(pb)
                while pending:
                    drip()
                pending = bundles.pop(pb)
        prev = (b, q, h_sb)
        # loads issued after the dumps so a buffer-gated load never
        # head-blocks them on the sync queue
        if step + PREFETCH < NSTEP:
            nb, nq = divmod(step + PREFETCH, NQ)
            load_stacks(nb, nq)
            if nq == 0:
                load_xew(nb)
    pb, pq, ph = prev
    mlp2_part(pb, pq, ph, drip)
    transpose_dx(pb)
    while pending:
        drip()
    pending = bundles.pop(pb)
    while pending:
        drip()


# ----------------------------------------------------------------------------
_PROGRAM_CACHE = {}


def _get_program():
    key = (LRELU_MODE,)
    if key in _PROGRAM_CACHE:
        return _PROGRAM_CACHE[key]
    nc = bacc.Bacc("TRN2", target_bir_lowering=False, debug=False,
                   num_devices=N_CORES)
    xrep1_in = nc.dram_tensor("xrep1_in", [B_LOC, 128, RT], BF16,
                              kind="ExternalInput").ap()
    xrep2_in = nc.dram_tensor("xrep2_in", [B_LOC, 128, RT], BF16,
                              kind="ExternalInput").ap()
    xrep3_in = nc.dram_tensor("xrep3_in", [B_LOC, 49, RT], BF16,
                              kind="ExternalInput").ap()
    xew_in = nc.dram_tensor("xew_in", [B_LOC, H, CW], F32,
                            kind="ExternalInput").ap()
    m_in = nc.dram_tensor("m_in", [H, B_LOC * W], F32,
                          kind="ExternalInput").ap()
    wa1_in = nc.dram_tensor("wa1_in", [128, HID], BF16,
                            kind="ExternalInput").ap()
    wa2_in = nc.dram_tensor("wa2_in", [128, HID], BF16,
                            kind="ExternalInput").ap()
    wa4_in = nc.dram_tensor("wa4_in", [113, HID], BF16,
                            kind="ExternalInput").ap()
    w2_in = nc.dram_tensor("w2_in", [HID, 32], F32, kind="ExternalInput").ap()
    b2_in = nc.dram_tensor("b2_in", [HID, 1], F32, kind="ExternalInput").ap()
    out_dram = nc.dram_tensor("out", [B_LOC, H, CW], F32,
                              kind="ExternalOutput").ap()
    scr_drams = [nc.dram_tensor(f"dxscr{b}", [H, C, W], F32).ap()
                 for b in range(B_LOC)]
    with tile.TileContext(nc) as tc:
        _build_kernel(tc, xrep1_in, xrep2_in, xrep3_in, xew_in, m_in,
                      wa1_in, wa2_in, wa4_in, w2_in, b2_in, out_dram,
                      scr_drams)
    nc.compile()
    _PROGRAM_CACHE[key] = nc
    return nc


def _host_weights(filters, W1, b1, W2, b2):
    import ml_dtypes
    filters = np.asarray(filters, np.float32)
    W1 = np.asarray(W1, np.float32)
    W2 = np.asarray(W2, np.float32)
    b1 = np.asarray(b1, np.float32)
    b2 = np.asarray(b2, np.float32)
    # Weff[o, c, d, dj] = sum_f W1[o, c*NF+f] * filters[f, d, dj]
    w1r = W1.reshape(HID, C, NF)
    weff = np.einsum("ocf,fij->ocij", w1r, filters)       # [o, c, d, dj]
    wah = weff.astype(ml_dtypes.bfloat16)
    wal = (weff - wah.astype(np.float32)).astype(ml_dtypes.bfloat16)
    wa1 = np.zeros((128, HID), ml_dtypes.bfloat16)
    wa2 = np.zeros((128, HID), ml_dtypes.bfloat16)
    for k, (dj, d) in enumerate(COMBOS[:8]):
        wa1[k * C:(k + 1) * C, :] = wah[:, :, d, dj].T
        wa2[k * C:(k + 1) * C, :] = wal[:, :, d, dj].T
    wa4 = np.zeros((113, HID), ml_dtypes.bfloat16)
    wa4[0:C, :] = wah[:, :, 2, 2].T
    wa4[C:2 * C, :] = wal[:, :, 2, 2].T
    wa4[2 * C:3 * C, :] = wah[:, :, 2, 2].T
    wa4[48, :] = b1.astype(ml_dtypes.bfloat16)
    wa4[64:113, :] = wa4[0:49, :]
    w2p = np.zeros((HID, 32), np.float32)
    w2p[:, :C] = W2.T
    b2v = np.zeros((HID, 1), np.float32)
    for j in range(4):
        b2v[32 * j:32 * j + C, 0] = b2
    return wa1, wa2, wa4, w2p, b2v


def _prepare_in_maps(x, rand_mask, filters, W1, b1, W2, b2):
    import ml_dtypes
    x = np.asarray(x, np.float32)
    # replica planes: xp rows/cols -1..128 (wrap); combo (dj,d) plane =
    # xp[:, :, d:d+128, dj:dj+128] == x[c, i+d-1, j+dj-1]
    xp = np.pad(x, ((0, 0), (0, 0), (1, 1), (1, 1)), mode="wrap")
    xp_h = xp.astype(ml_dtypes.bfloat16)
    xp_l = (xp - xp_h.astype(np.float32)).astype(ml_dtypes.bfloat16)
    xrep1 = np.empty((B, 8, C, H, W), ml_dtypes.bfloat16)
    xrep2 = np.empty((B, 8, C, H, W), ml_dtypes.bfloat16)
    for k, (dj, d) in enumerate(COMBOS[:8]):
        xrep1[:, k] = xp_h[:, :, d:d + H, dj:dj + W]
        xrep2[:, k] = xp_l[:, :, d:d + H, dj:dj + W]
    xrep1 = xrep1.reshape(B, 128, RT)
    xrep2 = xrep2.reshape(B, 128, RT)
    xrep3 = np.empty((B, 49, H, W), ml_dtypes.bfloat16)
    xrep3[:, 0:C] = xp_h[:, :, 2:2 + H, 2:2 + W]
    xrep3[:, C:2 * C] = xrep3[:, 0:C]
    xrep3[:, 2 * C:3 * C] = xp_l[:, :, 2:2 + H, 2:2 + W]
    xrep3[:, 48] = np.ones((B, H, W), ml_dtypes.bfloat16)
    xrep3 = xrep3.reshape(B, 49, RT)
    xew = np.ascontiguousarray(x.transpose(0, 2, 1, 3).reshape(B, H, CW))
    m = (np.asarray(rand_mask, np.float32)
         <= np.float32(FIRE_RATE)).astype(np.float32)
    m = m.reshape(B, H, W).transpose(1, 0, 2)   # [H, B, W]
    wa1, wa2, wa4, w2p, b2v = _host_weights(filters, W1, b1, W2, b2)
    in_maps = []
    for core in range(N_CORES):
        sl = slice(core * B_LOC, (core + 1) * B_LOC)
        in_maps.append({
            "xrep1_in": xrep1[sl], "xrep2_in": xrep2[sl],
            "xrep3_in": xrep3[sl], "xew_in": xew[sl],
            "m_in": np.ascontiguousarray(m[:, sl, :]).reshape(H, B_LOC * W),
            "wa1_in": wa1, "wa2_in": wa2, "wa4_in": wa4,
            "w2_in": w2p, "b2_in": b2v,
        })
    return in_maps


def kernel(x, rand_mask, filters, W1, b1, W2, b2, _want_trace=False):
    in_maps = _prepare_in_maps(x, rand_mask, filters, W1, b1, W2, b2)
    nc = _get_program()
    res = run_bass_kernel_spmd(nc, in_maps, list(range(N_CORES)),
                               trace=_want_trace)
    out = np.concatenate([res.results[i]["out"] for i in range(N_CORES)],
                         axis=0)
    out = np.ascontiguousarray(
        out.reshape(B, H, C, W).transpose(0, 2, 1, 3))
    if _want_trace:
        return out, res
    return out


# revision 55
# speedup vs baseline: 1.0349x; 1.0039x over previous
"""Trainium2 Bass kernel for one neural-CA (NCA) update step.

Model (per batch element, all f32):
  pre_life  = living_mask(x)                        # 3x3 circular max/avg pools on alpha=x[:,3]
  y         = depthwise 3x3 circular conv of x with 4 filters  -> [C*4, H, W]
  h         = leaky_relu(W1 @ y + b1, 0.01)         # per-pixel MLP, HID=128
  dx        = W2 @ h + b2
  xnew      = x + dx * (rand_mask <= 0.5)
  post_life = living_mask(xnew)
  out       = xnew * (pre_life & post_life)

Strategy (8 NeuronCores, pure data parallel over batch 32 -> 4 per core):
  * Fold conv+W1 into effective weights Weff[o, c, d, dj] (host precompute),
    split hi/lo in bf16: W x ~ Wh xh + Wl xh + Wh xl.
  * Both 3x3 shifts (d row, dj col) are BAKED into replica stacks: per
    quarter (32 image rows) three SBUF stacks hold the 9 (dj,d) shift
    combos x 16 channels:
      T1 [128, 4096] = xh for combos 0-7   (partition p = dj*48 + d*16 + c)
      T2 [128, 4096] = xl for combos 0-7
      T3 [49, 4096]  = [xh c8; xh c8; xl c8; ones]
    so conv+MLP1 for a 512-px chunk is 4 back-to-back matmuls (K=128,128,
    128,49) with contiguous N=512 rhs slices; b1 rides the ones row.
  * Weight-stationary: cfg-outer loop over the 8 chunks of a quarter ->
    weights swap 4x per 32 matmuls instead of per-matmul; PSUM = 4 pair
    tiles [128, 1024] (2 banks each, all 8 banks).
  * lrelu evac per pair tile, alternating ScalarE (HW Lrelu) / VectorE
    (mult+max decomp) to balance engines.
  * MLP2 (K=128, M=16->32 zero-padded, fp32) col-tiled 4x via
    tile_position; DVE evacuates dx (+b2) into a per-batch c-major tile.
  * dx transposed to H-major [H, C*W] via SBUF->SBUF DMA (no DRAM bounce);
    elementwise tail + life-mask pools run there with 128-partition tiles,
    dripped between the next batch's quarters.
"""

import os
import sys

os.environ.setdefault("JAX_PLATFORMS", "cpu")
for _p in ("/opt/trn_rl_repo", "/root/.axon_site/_ro/trn_rl_repo"):
    if os.path.isdir(_p) and _p not in sys.path:
        sys.path.insert(0, _p)

from contextlib import ExitStack

import numpy as np

import concourse.bass as bass
import concourse.tile as tile
from concourse import bacc, mybir
from concourse._compat import with_exitstack
from concourse.bass_utils import run_bass_kernel_spmd

# ----------------------------------------------------------------------------
# problem constants (hardcoded per spec nn_CAModel_2121713844629)
B, C, H, W = 32, 16, 128, 128
NF, R, K = 4, 1, 3
HID = 128
FIRE_RATE = 0.5
NEG_SLOPE = 0.01
N_CORES = 8
B_LOC = B // N_CORES          # 4 batches per core
CW = C * W                    # 2048
SW = W + 2                    # 130 padded row width (cols -1..128)
RT = H * W                    # 16384, per-channel replica plane size
QROWS = 16                    # image rows per step ("octant")
NQ = H // QROWS               # 8 octants per batch
QF = QROWS * W                # 2048 octant free size
LQF = 2 * QF                  # 4096: stacks are LOADED per 32-row quarter
CPQ = 4                       # 512-px chunks per octant
CHUNK = 512
NPAIR = 2                     # psum pair tiles per octant (2 chunks each)

LRELU_MODE = os.environ.get("CA_LRELU", "act")   # "act" (HW Lrelu) / "decomp"

F32 = mybir.dt.float32
BF16 = mybir.dt.bfloat16

# combo order: k = dj*3 + d ; combos 0-7 in T1/T2, combo 8 = (dj=2,d=2) in T3
COMBOS = [(dj, d) for dj in range(3) for d in range(3)]


def _avg_threshold():
    """Smallest f32 s with (np.float32(s)/9 < 0.2) False, as the strict-< bound."""
    lo = np.float32(1.7)
    hi = np.float32(1.9)
    for _ in range(80):
        mid = np.float32((lo.astype(np.float64) + hi.astype(np.float64)) / 2)
        if mid / np.float32(9.0) < np.float32(0.2):
            lo = mid
        else:
            hi = mid
    return float(hi)


AVG_LT = _avg_threshold()


# ----------------------------------------------------------------------------
@with_exitstack
def _build_kernel(ctx: ExitStack, tc: "tile.TileContext",
                  xrep1_in, xrep2_in, xrep3_in, xew_in, m_in,
                  wa1_in, wa2_in, wa4_in, w2_in, b2_in, out_dram, scr_drams):
    nc = tc.nc
    consts = ctx.enter_context(tc.tile_pool(name="consts", bufs=1))
    st1 = ctx.enter_context(tc.tile_pool(name="st1", bufs=2))
    st2 = ctx.enter_context(tc.tile_pool(name="st2", bufs=2))
    st3 = ctx.enter_context(tc.tile_pool(name="st3", bufs=2))
    hpool = ctx.enter_context(tc.tile_pool(name="hpool", bufs=6))
    dxcpool = ctx.enter_context(tc.tile_pool(name="dxc", bufs=3))
    ewpool = ctx.enter_context(tc.tile_pool(name="ewpool", bufs=2))
    small = ctx.enter_context(tc.tile_pool(name="small", bufs=1))
    psum = ctx.enter_context(tc.tile_pool(name="psum", bufs=4, space="PSUM"))

    # --- constants (weights on the scalar queue so the first stack loads
    # lead the sync queue; m_all last — phase B only) ------------------------
    wa1_t = consts.tile([128, HID], BF16)
    wa2_t = consts.tile([128, HID], BF16)
    wa4_t = consts.tile([113, HID], BF16)
    w2_t = consts.tile([HID, 32], F32)
    b2_t = consts.tile([HID, 1], F32)
    m_all = consts.tile([H, B_LOC * W], F32)

    def load_consts():
        nc.scalar.dma_start(wa1_t[:], wa1_in[:])
        nc.scalar.dma_start(wa2_t[:], wa2_in[:])
        nc.scalar.dma_start(wa4_t[:], wa4_in[:])
        nc.scalar.dma_start(w2_t[:], w2_in[:])
        nc.scalar.dma_start(b2_t[:], b2_in[:])

    state = {}

    def load_stacks(b, lq):
        """Issue stack loads for (batch b, 32-row load-quarter lq) —
        host-baked replicas, one 1MB contiguous-per-partition DMA per tile
        (bigger transfers keep the SDMA near peak bandwidth). All on the
        sync queue: the scalar queue carries ACT evacs."""
        t1 = st1.tile([128, LQF], BF16, name=f"t1_{b}_{lq}", tag="t1")
        t2 = st2.tile([128, LQF], BF16, name=f"t2_{b}_{lq}", tag="t2")
        t3 = st3.tile([49, LQF], BF16, name=f"t3_{b}_{lq}", tag="t3")
        for src_dram, dst, npart in ((xrep1_in, t1, 128),
                                     (xrep2_in, t2, 128),
                                     (xrep3_in, t3, 49)):
            srcap = bass.AP(
                tensor=src_dram.tensor,
                offset=src_dram.offset + (b * npart) * RT + lq * LQF,
                ap=[[RT, npart], [1, LQF]])
            nc.sync.dma_start(dst[0:npart, :], srcap)
        state["stacks", b, lq] = (t1, t2, t3)

    def mlp1_part(b, q, drip):
        """MLP1 MMs + lrelu evacs for octant (b, q); returns h_sb pair."""
        if q % 2 == 1:
            t1, t2, t3 = state.pop(("stacks", b, q // 2))
        else:
            t1, t2, t3 = state[("stacks", b, q // 2)]
        off = (q % 2) * QF
        hps = [psum.tile([HID, 2 * CHUNK], F32, name=f"hps{b}_{q}_{p}",
                         tag="hps") for p in range(NPAIR)]

        def rhs(st, cl, kp):
            return bass.AP(tensor=st.tensor,
                           offset=st.offset + off + cl * CHUNK,
                           ap=[[LQF, kp], [1, CHUNK]])

        cfgs = ((wa1_t, t1, 128), (wa1_t, t2, 128),
                (wa2_t, t1, 128), (wa4_t, t3, 49))

        def evac(p):
            """lrelu evac of pair p: p0 on ScalarE, p1 on VectorE — safe to
            share DVE with phase-B thunks now that MLP2 consumption is
            deferred a full octant."""
            hs = hpool.tile([HID, 2 * CHUNK], F32, name=f"h{b}_{q}_{p}",
                            tag="h_sb")
            if LRELU_MODE == "act" and p == 0:
                nc.scalar.activation(hs[:], hps[p][:],
                                     mybir.ActivationFunctionType.Lrelu,
                                     scale=1.0, alpha=NEG_SLOPE)
            else:
                tt = small.tile([HID, 2 * CHUNK], F32, name=f"lt{b}_{q}_{p}",
                                tag="ltmp", bufs=2)
                nc.vector.tensor_scalar(tt[:], hps[p][:], NEG_SLOPE, None,
                                        op0=mybir.AluOpType.mult)
                nc.vector.tensor_tensor(hs[:], hps[p][:], tt[:],
                                        op=mybir.AluOpType.max)
            return hs

        h_sb = [None] * NPAIR
        for ci, (wt, st, kp) in enumerate(cfgs):
            for cl in range(CPQ):
                dst = hps[cl // 2][:, (cl % 2) * CHUNK:
                                   (cl % 2 + 1) * CHUNK]
                nc.tensor.matmul(dst, wt[0:kp, :], rhs(st, cl, kp),
                                 start=(ci == 0), stop=(ci == 3))
        for p in range(NPAIR):
            h_sb[p] = evac(p)
        drip()
        return h_sb

    def mlp2_part(b, q, h_sb, drip):
        """MLP2 + dx evac + dumps for octant (b, q) — issued one octant
        late so the in-order PE queue never waits on the lrelu evacs."""
        # MLP2: one group of 4 chunks, col-tiled fp32
        dxp = psum.tile([HID, CHUNK], F32, name=f"dxp{b}_{q}", tag="hps")
        for j in range(4):
            nc.tensor.matmul(
                dxp[32 * j:32 * j + 32, :],
                w2_t[:, 0:32],
                h_sb[j // 2][:, (j % 2) * CHUNK:(j % 2 + 1) * CHUNK],
                start=True, stop=True, tile_position=(0, 32 * j))
        # dx evac + b2 into c-major staging (ScalarE), then dump H-major
        # into the DRAM scratch (the partition<->free transpose can only
        # happen through a DRAM-side free-form AP)
        dxq = dxcpool.tile([HID, CHUNK], F32, name=f"dxq{b}_{q}",
                           tag="dxq")
        nc.scalar.activation(dxq[:], dxp[:],
                             mybir.ActivationFunctionType.Identity,
                             bias=b2_t[:], scale=1.0)
        scr = scr_drams[b]
        for j in range(4):
            eng = nc.gpsimd if j < 2 else nc.sync
            srcap = bass.AP(
                tensor=dxq.tensor,
                offset=dxq.offset + (32 * j) * CHUNK,
                ap=[[CHUNK, C], [W, 4], [1, W]])
            dstap = bass.AP(
                tensor=scr.tensor,
                offset=scr.offset + (16 * q + 4 * j) * CW,
                ap=[[W, C], [CW, 4], [1, W]])
            eng.dma_start(dstap, srcap)
        drip()

    def transpose_dx(b):
        """dxc [32j+c, g*512+r*128+w] -> dx_ew [16g+4j+r, c*128+w]."""
        dx_ew = ewpool.tile([H, CW], F32, name=f"dxew{b}", tag="dx_ew")
        nc.gpsimd.dma_start(dx_ew[:],
                            scr_drams[b].rearrange("h c w -> h (c w)"))
        state["dx_ew", b] = dx_ew

    def load_xew(b):
        x_ew = ewpool.tile([H, CW], F32, name=f"xew{b}", tag="x_ew", bufs=3)
        nc.gpsimd.dma_start(x_ew[:], xew_in[b])
        state["x_ew", b] = x_ew

    def phase_B_bundles(b):
        """elementwise tail + life masks + store for batch b (thunk list)."""
        x_ew = state.pop(("x_ew", b))
        st = {}

        def bcast(t128):
            return bass.AP(tensor=t128.tensor, offset=t128.offset,
                           ap=[[t128.ap[0][0], H], [0, C], [1, W]])

        def bn_ew():
            dx_ew = state.pop(("dx_ew", b))
            m_b = bass.AP(tensor=m_all.tensor, offset=m_all.offset + b * W,
                          ap=[[m_all.ap[0][0], H], [0, C], [1, W]])
            nc.vector.tensor_tensor(dx_ew[:], dx_ew[:], m_b,
                                    op=mybir.AluOpType.mult)
            xnew = ewpool.tile([H, CW], F32, name=f"xnew{b}", tag="xnew")
            nc.vector.tensor_tensor(xnew[:], x_ew[:], dx_ew[:],
                                    op=mybir.AluOpType.add)
            st["xnew"] = xnew

        def living(src_getter, which):
            def fn():
                src_ew = src_getter()
                ap_pad = small.tile([H, SW], F32, name=f"ap{which}{b}",
                                    tag=f"ap{which}")
                alpha = src_ew[:, 3 * W:4 * W]
                nc.vector.tensor_copy(ap_pad[:, 1:1 + W], alpha)
                nc.vector.tensor_copy(ap_pad[:, 0:1],
                                      src_ew[:, 4 * W - 1:4 * W])
                nc.vector.tensor_copy(ap_pad[:, 1 + W:2 + W],
                                      src_ew[:, 3 * W:3 * W + 1])
                hh = small.tile([H, 2 * W], F32, name=f"hh{which}{b}",
                                tag=f"hh{which}")
                hm = hh[:, 0:W]
                hs = hh[:, W:2 * W]
                nc.vector.tensor_tensor(hm, ap_pad[:, 0:W], ap_pad[:, 1:1 + W],
                                        op=mybir.AluOpType.max)
                nc.vector.tensor_tensor(hm, hm, ap_pad[:, 2:2 + W],
                                        op=mybir.AluOpType.max)
                nc.vector.tensor_tensor(hs, ap_pad[:, 0:W], ap_pad[:, 1:1 + W],
                                        op=mybir.AluOpType.add)
                nc.vector.tensor_tensor(hs, hs, ap_pad[:, 2:2 + W],
                                        op=mybir.AluOpType.add)
                up = small.tile([H, 2 * W], F32, name=f"up{which}{b}",
                                tag=f"up{which}")
                dn = small.tile([H, 2 * W], F32, name=f"dn{which}{b}",
                                tag=f"dn{which}")
                nc.gpsimd.dma_start(up[0:H - 1, :], hh[1:H, :])
                nc.gpsimd.dma_start(up[H - 1:H, :], hh[0:1, :])
                nc.gpsimd.dma_start(dn[1:H, :], hh[0:H - 1, :])
                nc.gpsimd.dma_start(dn[0:1, :], hh[H - 1:H, :])
                st[f"hh{which}"] = (hh, up, dn)
            return fn

        def living_v(which):
            def fn():
                hh, up, dn = st.pop(f"hh{which}")
                vm = small.tile([H, W], F32, name=f"vm{which}{b}",
                                tag=f"vm{which}")
                vs = small.tile([H, W], F32, name=f"vs{which}{b}",
                                tag=f"vs{which}")
                for (t_out, o0, op) in ((vm, 0, mybir.AluOpType.max),
                                        (vs, W, mybir.AluOpType.add)):
                    nc.vector.tensor_tensor(t_out[:], hh[:, o0:o0 + W],
                                            up[:, o0:o0 + W], op=op)
                    nc.vector.tensor_tensor(t_out[:], t_out[:],
                                            dn[:, o0:o0 + W], op=op)
                alive = small.tile([H, W], F32, name=f"al{which}{b}",
                                   tag=f"al{which}")
                nc.vector.tensor_scalar(alive[:], vm[:], 0.1, None,
                                        op0=mybir.AluOpType.is_gt)
                avgok = small.tile([H, W], F32, name=f"ag{which}{b}",
                                   tag=f"ag{which}")
                nc.vector.tensor_scalar(avgok[:], vs[:], AVG_LT, None,
                                        op0=mybir.AluOpType.is_lt)
                lif = small.tile([H, W], F32, name=f"lf{which}{b}",
                                 tag=f"lf{which}")
                nc.vector.tensor_tensor(lif[:], alive[:], avgok[:],
                                        op=mybir.AluOpType.mult)
                st[f"life{which}"] = lif
            return fn

        def bn_final():
            xnew = st["xnew"]
            life = small.tile([H, W], F32, name=f"life{b}", tag="life")
            nc.vector.tensor_tensor(life[:], st["lifepre"][:],
                                    st["lifepost"][:],
                                    op=mybir.AluOpType.mult)
            nc.vector.tensor_tensor(xnew[:], xnew[:], bcast(life),
                                    op=mybir.AluOpType.mult)
            nc.sync.dma_start(out_dram[b], xnew[:])

        pre = [
            living(lambda: x_ew, "pre"),
            living_v("pre"),
        ]
        post = [
            bn_ew,
            living(lambda: st["xnew"], "post"),
            living_v("post"),
            bn_final,
        ]
        return pre, post

    # --- pipeline: (b, q) steps, dripping phase-B thunks between MM blocks:
    # pre-life of batch b runs during b's own quarters; the tail (xnew,
    # post-life, store) runs during b+1's quarters.
    pending = []

    def drip():
        if pending:
            pending.pop(0)()

    NSTEP = B_LOC * NQ
    PREFETCH = 4                  # octants of lookahead (= 2 load-quarters)
    bundles = {}
    load_stacks(0, 0)
    load_consts()
    load_stacks(0, 1)
    prev = None
    for step in range(NSTEP):
        b, q = divmod(step, NQ)
        if step == 1:
            # deferred past the startup burst: only needed by phase B
            load_xew(0)
            nc.gpsimd.dma_start(m_all[:], m_in[:])
        if q == 1:
            pre, post = phase_B_bundles(b)
            bundles[b] = post
            pending.extend(pre)
        h_sb = mlp1_part(b, q, drip)
        if prev is not None:
            pb, pq, ph = prev
            mlp2_part(pb, pq, ph, drip)
            if pq == NQ - 1:
                transpose_dx(pb)
                while pending:
                    drip()
                pending = bundles.pop(pb)
        prev = (b, q, h_sb)
        # loads issued after the dumps so a buffer-gated load never
        # head-blocks them on the sync queue
        if step + PREFETCH < NSTEP and (step + PREFETCH) % 2 == 0:
            nb, nq = divmod(step + PREFETCH, NQ)
            load_stacks(nb, nq // 2)
            if nq == 0:
                load_xew(nb)
    pb, pq, ph = prev
    mlp2_part(pb, pq, ph, drip)
    transpose_dx(pb)
    while pending:
        drip()
    pending = bundles.pop(pb)
    while pending:
        drip()


# ----------------------------------------------------------------------------
_PROGRAM_CACHE = {}


def _get_program():
    key = (LRELU_MODE,)
    if key in _PROGRAM_CACHE:
        return _PROGRAM_CACHE[key]
    nc = bacc.Bacc("TRN2", target_bir_lowering=False, debug=False,
                   num_devices=N_CORES)
    xrep1_in = nc.dram_tensor("xrep1_in", [B_LOC, 128, RT], BF16,
                              kind="ExternalInput").ap()
    xrep2_in = nc.dram_tensor("xrep2_in", [B_LOC, 128, RT], BF16,
                              kind="ExternalInput").ap()
    xrep3_in = nc.dram_tensor("xrep3_in", [B_LOC, 49, RT], BF16,
                              kind="ExternalInput").ap()
    xew_in = nc.dram_tensor("xew_in", [B_LOC, H, CW], F32,
                            kind="ExternalInput").ap()
    m_in = nc.dram_tensor("m_in", [H, B_LOC * W], F32,
                          kind="ExternalInput").ap()
    wa1_in = nc.dram_tensor("wa1_in", [128, HID], BF16,
                            kind="ExternalInput").ap()
    wa2_in = nc.dram_tensor("wa2_in", [128, HID], BF16,
                            kind="ExternalInput").ap()
    wa4_in = nc.dram_tensor("wa4_in", [113, HID], BF16,
                            kind="ExternalInput").ap()
    w2_in = nc.dram_tensor("w2_in", [HID, 32], F32, kind="ExternalInput").ap()
    b2_in = nc.dram_tensor("b2_in", [HID, 1], F32, kind="ExternalInput").ap()
    out_dram = nc.dram_tensor("out", [B_LOC, H, CW], F32,
                              kind="ExternalOutput").ap()
    scr_drams = [nc.dram_tensor(f"dxscr{b}", [H, C, W], F32).ap()
                 for b in range(B_LOC)]
    with tile.TileContext(nc) as tc:
        _build_kernel(tc, xrep1_in, xrep2_in, xrep3_in, xew_in, m_in,
                      wa1_in, wa2_in, wa4_in, w2_in, b2_in, out_dram,
                      scr_drams)
    nc.compile()
    _PROGRAM_CACHE[key] = nc
    return nc


def _host_weights(filters, W1, b1, W2, b2):
    import ml_dtypes
    filters = np.asarray(filters, np.float32)
    W1 = np.asarray(W1, np.float32)
    W2 = np.asarray(W2, np.float32)
    b1 = np.asarray(b1, np.float32)
    b2 = np.asarray(b2, np.float32)
    # Weff[o, c, d, dj] = sum_f W1[o, c*NF+f] * filters[f, d, dj]
    w1r = W1.reshape(HID, C, NF)
    weff = np.einsum("ocf,fij->ocij", w1r, filters)       # [o, c, d, dj]
    wah = weff.astype(ml_dtypes.bfloat16)
    wal = (weff - wah.astype(np.float32)).astype(ml_dtypes.bfloat16)
    wa1 = np.zeros((128, HID), ml_dtypes.bfloat16)
    wa2 = np.zeros((128, HID), ml_dtypes.bfloat16)
    for k, (dj, d) in enumerate(COMBOS[:8]):
        wa1[k * C:(k + 1) * C, :] = wah[:, :, d, dj].T
        wa2[k * C:(k + 1) * C, :] = wal[:, :, d, dj].T
    wa4 = np.zeros((113, HID), ml_dtypes.bfloat16)
    wa4[0:C, :] = wah[:, :, 2, 2].T
    wa4[C:2 * C, :] = wal[:, :, 2, 2].T
    wa4[2 * C:3 * C, :] = wah[:, :, 2, 2].T
    wa4[48, :] = b1.astype(ml_dtypes.bfloat16)
    wa4[64:113, :] = wa4[0:49, :]
    w2p = np.zeros((HID, 32), np.float32)
    w2p[:, :C] = W2.T
    b2v = np.zeros((HID, 1), np.float32)
    for j in range(4):
        b2v[32 * j:32 * j + C, 0] = b2
    return wa1, wa2, wa4, w2p, b2v


def _prepare_in_maps(x, rand_mask, filters, W1, b1, W2, b2):
    import ml_dtypes
    x = np.asarray(x, np.float32)
    # replica planes: xp rows/cols -1..128 (wrap); combo (dj,d) plane =
    # xp[:, :, d:d+128, dj:dj+128] == x[c, i+d-1, j+dj-1]
    xp = np.pad(x, ((0, 0), (0, 0), (1, 1), (1, 1)), mode="wrap")
    xp_h = xp.astype(ml_dtypes.bfloat16)
    xp_l = (xp - xp_h.astype(np.float32)).astype(ml_dtypes.bfloat16)
    xrep1 = np.empty((B, 8, C, H, W), ml_dtypes.bfloat16)
    xrep2 = np.empty((B, 8, C, H, W), ml_dtypes.bfloat16)
    for k, (dj, d) in enumerate(COMBOS[:8]):
        xrep1[:, k] = xp_h[:, :, d:d + H, dj:dj + W]
        xrep2[:, k] = xp_l[:, :, d:d + H, dj:dj + W]
    xrep1 = xrep1.reshape(B, 128, RT)
    xrep2 = xrep2.reshape(B, 128, RT)
    xrep3 = np.empty((B, 49, H, W), ml_dtypes.bfloat16)
    xrep3[:, 0:C] = xp_h[:, :, 2:2 + H, 2:2 + W]
    xrep3[:, C:2 * C] = xrep3[:, 0:C]
    xrep3[:, 2 * C:3 * C] = xp_l[:, :, 2:2 + H, 2:2 + W]
    xrep3[:, 48] = np.ones((B, H, W), ml_dtypes.bfloat16)
    xrep3 = xrep3.reshape(B, 49, RT)
    xew = np.ascontiguousarray(x.transpose(0, 2, 1, 3).reshape(B, H, CW))
    m = (np.asarray(rand_mask, np.float32)
         <= np.float32(FIRE_RATE)).astype(np.float32)
    m = m.reshape(B, H, W).transpose(1, 0, 2)   # [H, B, W]
    wa1, wa2, wa4, w2p, b2v = _host_weights(filters, W1, b1, W2, b2)
    in_maps = []
    for core in range(N_CORES):
        sl = slice(core * B_LOC, (core + 1) * B_LOC)
        in_maps.append({
            "xrep1_in": xrep1[sl], "xrep2_in": xrep2[sl],
            "xrep3_in": xrep3[sl], "xew_in": xew[sl],
            "m_in": np.ascontiguousarray(m[:, sl, :]).reshape(H, B_LOC * W),
            "wa1_in": wa1, "wa2_in": wa2, "wa4_in": wa4,
            "w2_in": w2p, "b2_in": b2v,
        })
    return in_maps


def kernel(x, rand_mask, filters, W1, b1, W2, b2, _want_trace=False):
    in_maps = _prepare_in_maps(x, rand_mask, filters, W1, b1, W2, b2)
    nc = _get_program()
    res = run_bass_kernel_spmd(nc, in_maps, list(range(N_CORES)),
                               trace=_want_trace)
    out = np.concatenate([res.results[i]["out"] for i in range(N_CORES)],
                         axis=0)
    out = np.ascontiguousarray(
        out.reshape(B, H, C, W).transpose(0, 2, 1, 3))
    if _want_trace:
        return out, res
    return out


# revision 56
# speedup vs baseline: 1.1033x; 1.0661x over previous
"""Trainium2 Bass kernel for one neural-CA (NCA) update step.

Model (per batch element, all f32):
  pre_life  = living_mask(x)                        # 3x3 circular max/avg pools on alpha=x[:,3]
  y         = depthwise 3x3 circular conv of x with 4 filters  -> [C*4, H, W]
  h         = leaky_relu(W1 @ y + b1, 0.01)         # per-pixel MLP, HID=128
  dx        = W2 @ h + b2
  xnew      = x + dx * (rand_mask <= 0.5)
  post_life = living_mask(xnew)
  out       = xnew * (pre_life & post_life)

Strategy (8 NeuronCores, pure data parallel over batch 32 -> 4 per core):
  * Fold conv+W1 into effective weights Weff[o, c, d, dj] (host precompute),
    split hi/lo in bf16: W x ~ Wh xh + Wl xh + Wh xl.
  * Both 3x3 shifts (d row, dj col) are BAKED into replica stacks: per
    quarter (32 image rows) three SBUF stacks hold the 9 (dj,d) shift
    combos x 16 channels:
      T1 [128, 4096] = xh for combos 0-7   (partition p = dj*48 + d*16 + c)
      T2 [128, 4096] = xl for combos 0-7
      T3 [49, 4096]  = [xh c8; xh c8; xl c8; ones]
    so conv+MLP1 for a 512-px chunk is 4 back-to-back matmuls (K=128,128,
    128,49) with contiguous N=512 rhs slices; b1 rides the ones row.
  * Weight-stationary: cfg-outer loop over the 8 chunks of a quarter ->
    weights swap 4x per 32 matmuls instead of per-matmul; PSUM = 4 pair
    tiles [128, 1024] (2 banks each, all 8 banks).
  * lrelu evac per pair tile, alternating ScalarE (HW Lrelu) / VectorE
    (mult+max decomp) to balance engines.
  * MLP2 (K=128, M=16->32 zero-padded, fp32) col-tiled 4x via
    tile_position; DVE evacuates dx (+b2) into a per-batch c-major tile.
  * dx transposed to H-major [H, C*W] via SBUF->SBUF DMA (no DRAM bounce);
    elementwise tail + life-mask pools run there with 128-partition tiles,
    dripped between the next batch's quarters.
"""

import os
import sys

os.environ.setdefault("JAX_PLATFORMS", "cpu")
for _p in ("/opt/trn_rl_repo", "/root/.axon_site/_ro/trn_rl_repo"):
    if os.path.isdir(_p) and _p not in sys.path:
        sys.path.insert(0, _p)

from contextlib import ExitStack

import numpy as np

import concourse.bass as bass
import concourse.tile as tile
from concourse import bacc, mybir
from concourse._compat import with_exitstack
from concourse.bass_utils import run_bass_kernel_spmd

# ----------------------------------------------------------------------------
# problem constants (hardcoded per spec nn_CAModel_2121713844629)
B, C, H, W = 32, 16, 128, 128
NF, R, K = 4, 1, 3
HID = 128
FIRE_RATE = 0.5
NEG_SLOPE = 0.01
N_CORES = 8
B_LOC = B // N_CORES          # 4 batches per core
CW = C * W                    # 2048
SW = W + 2                    # 130 padded row width (cols -1..128)
RT = H * W                    # 16384, per-channel replica plane size
QROWS = 16                    # image rows per step ("octant")
NQ = H // QROWS               # 8 octants per batch
QF = QROWS * W                # 2048 octant free size
LQF = 2 * QF                  # 4096: stacks are LOADED per 32-row quarter
CPQ = 4                       # 512-px chunks per octant
CHUNK = 512
NPAIR = 2                     # psum pair tiles per octant (2 chunks each)

LRELU_MODE = os.environ.get("CA_LRELU", "act")   # "act" (HW Lrelu) / "decomp"

F32 = mybir.dt.float32
BF16 = mybir.dt.bfloat16

# combo order: k = dj*3 + d ; combos 0-7 in T1/T2, combo 8 = (dj=2,d=2) in T3
COMBOS = [(dj, d) for dj in range(3) for d in range(3)]


def _avg_threshold():
    """Smallest f32 s with (np.float32(s)/9 < 0.2) False, as the strict-< bound."""
    lo = np.float32(1.7)
    hi = np.float32(1.9)
    for _ in range(80):
        mid = np.float32((lo.astype(np.float64) + hi.astype(np.float64)) / 2)
        if mid / np.float32(9.0) < np.float32(0.2):
            lo = mid
        else:
            hi = mid
    return float(hi)


AVG_LT = _avg_threshold()


# ----------------------------------------------------------------------------
@with_exitstack
def _build_kernel(ctx: ExitStack, tc: "tile.TileContext",
                  xrep1_in, xrep2_in, xrep3_in, xew_in, m_in,
                  wa1_in, wa2_in, wa4_in, w2_in, b2_in, out_dram, scr_drams):
    nc = tc.nc
    consts = ctx.enter_context(tc.tile_pool(name="consts", bufs=1))
    st1 = ctx.enter_context(tc.tile_pool(name="st1", bufs=2))
    st2 = ctx.enter_context(tc.tile_pool(name="st2", bufs=2))
    st3 = ctx.enter_context(tc.tile_pool(name="st3", bufs=2))
    hpool = ctx.enter_context(tc.tile_pool(name="hpool", bufs=6))
    dxcpool = ctx.enter_context(tc.tile_pool(name="dxc", bufs=3))
    ewpool = ctx.enter_context(tc.tile_pool(name="ewpool", bufs=2))
    small = ctx.enter_context(tc.tile_pool(name="small", bufs=1))
    psum = ctx.enter_context(tc.tile_pool(name="psum", bufs=4, space="PSUM"))

    # --- constants (weights on the scalar queue so the first stack loads
    # lead the sync queue; m_all last — phase B only) ------------------------
    wa1_t = consts.tile([128, HID], BF16)
    wa2_t = consts.tile([128, HID], BF16)
    wa4_t = consts.tile([113, HID], BF16)
    w2_t = consts.tile([HID, 32], F32)
    b2_t = consts.tile([HID, 1], F32)
    m_all = consts.tile([H, B_LOC * W], F32)

    def load_consts():
        nc.scalar.dma_start(wa1_t[:], wa1_in[:])
        nc.scalar.dma_start(wa2_t[:], wa2_in[:])
        nc.scalar.dma_start(wa4_t[:], wa4_in[:])
        nc.scalar.dma_start(w2_t[:], w2_in[:])
        nc.scalar.dma_start(b2_t[:], b2_in[:])

    state = {}

    def load_stacks(b, lq):
        """Issue stack loads for (batch b, 32-row load-quarter lq) —
        host-baked replicas, one 1MB contiguous-per-partition DMA per tile
        (bigger transfers keep the SDMA near peak bandwidth). All on the
        sync queue: the scalar queue carries ACT evacs."""
        t1 = st1.tile([128, LQF], BF16, name=f"t1_{b}_{lq}", tag="t1")
        t2 = st2.tile([128, LQF], BF16, name=f"t2_{b}_{lq}", tag="t2")
        t3 = st3.tile([49, LQF], BF16, name=f"t3_{b}_{lq}", tag="t3")
        for src_dram, dst, npart in ((xrep1_in, t1, 128),
                                     (xrep2_in, t2, 128),
                                     (xrep3_in, t3, 49)):
            srcap = bass.AP(
                tensor=src_dram.tensor,
                offset=src_dram.offset + (b * npart) * RT + lq * LQF,
                ap=[[RT, npart], [1, LQF]])
            nc.sync.dma_start(dst[0:npart, :], srcap)
        state["stacks", b, lq] = (t1, t2, t3)

    def mlp1_part(b, q, drip):
        """MLP1 MMs + lrelu evacs for octant (b, q); returns h_sb pair."""
        if q % 2 == 1:
            t1, t2, t3 = state.pop(("stacks", b, q // 2))
        else:
            t1, t2, t3 = state[("stacks", b, q // 2)]
        off = (q % 2) * QF
        hps = [psum.tile([HID, 2 * CHUNK], F32, name=f"hps{b}_{q}_{p}",
                         tag="hps") for p in range(NPAIR)]

        def rhs(st, cl, kp):
            return bass.AP(tensor=st.tensor,
                           offset=st.offset + off + cl * CHUNK,
                           ap=[[LQF, kp], [1, CHUNK]])

        cfgs = ((wa1_t, t1, 128), (wa1_t, t2, 128),
                (wa2_t, t1, 128), (wa4_t, t3, 49))

        def evac(p):
            """lrelu evac — ScalarE only; DVE belongs to phase B (its long
            dep chains head-block anything queued behind them), and the
            one-octant MLP2 deferral gives the ACT chain consumer slack."""
            hs = hpool.tile([HID, 2 * CHUNK], F32, name=f"h{b}_{q}_{p}",
                            tag="h_sb")
            if LRELU_MODE == "act":
                nc.scalar.activation(hs[:], hps[p][:],
                                     mybir.ActivationFunctionType.Lrelu,
                                     scale=1.0, alpha=NEG_SLOPE)
            else:
                tt = small.tile([HID, 2 * CHUNK], F32, name=f"lt{b}_{q}_{p}",
                                tag="ltmp", bufs=2)
                nc.vector.tensor_scalar(tt[:], hps[p][:], NEG_SLOPE, None,
                                        op0=mybir.AluOpType.mult)
                nc.vector.tensor_tensor(hs[:], hps[p][:], tt[:],
                                        op=mybir.AluOpType.max)
            return hs

        h_sb = [None] * NPAIR
        for ci, (wt, st, kp) in enumerate(cfgs):
            for cl in range(CPQ):
                dst = hps[cl // 2][:, (cl % 2) * CHUNK:
                                   (cl % 2 + 1) * CHUNK]
                nc.tensor.matmul(dst, wt[0:kp, :], rhs(st, cl, kp),
                                 start=(ci == 0), stop=(ci == 3))
        for p in range(NPAIR):
            h_sb[p] = evac(p)
        drip()
        return h_sb

    def mlp2_part(b, q, h_sb, drip):
        """MLP2 + dx evac + dumps for octant (b, q) — issued one octant
        late so the in-order PE queue never waits on the lrelu evacs."""
        # MLP2: one group of 4 chunks, col-tiled fp32
        dxp = psum.tile([HID, CHUNK], F32, name=f"dxp{b}_{q}", tag="hps")
        for j in range(4):
            nc.tensor.matmul(
                dxp[32 * j:32 * j + 32, :],
                w2_t[:, 0:32],
                h_sb[j // 2][:, (j % 2) * CHUNK:(j % 2 + 1) * CHUNK],
                start=True, stop=True, tile_position=(0, 32 * j))
        # dx evac + b2 into c-major staging (ScalarE), then dump H-major
        # into the DRAM scratch (the partition<->free transpose can only
        # happen through a DRAM-side free-form AP)
        dxq = dxcpool.tile([HID, CHUNK], F32, name=f"dxq{b}_{q}",
                           tag="dxq")
        nc.scalar.activation(dxq[:], dxp[:],
                             mybir.ActivationFunctionType.Identity,
                             bias=b2_t[:], scale=1.0)
        scr = scr_drams[b]
        for j in range(4):
            eng = nc.gpsimd if j < 2 else nc.sync
            srcap = bass.AP(
                tensor=dxq.tensor,
                offset=dxq.offset + (32 * j) * CHUNK,
                ap=[[CHUNK, C], [W, 4], [1, W]])
            dstap = bass.AP(
                tensor=scr.tensor,
                offset=scr.offset + (16 * q + 4 * j) * CW,
                ap=[[W, C], [CW, 4], [1, W]])
            eng.dma_start(dstap, srcap)
        drip()

    def transpose_dx(b):
        """dxc [32j+c, g*512+r*128+w] -> dx_ew [16g+4j+r, c*128+w]."""
        dx_ew = ewpool.tile([H, CW], F32, name=f"dxew{b}", tag="dx_ew")
        nc.gpsimd.dma_start(dx_ew[:],
                            scr_drams[b].rearrange("h c w -> h (c w)"))
        state["dx_ew", b] = dx_ew

    def load_xew(b):
        x_ew = ewpool.tile([H, CW], F32, name=f"xew{b}", tag="x_ew", bufs=3)
        nc.gpsimd.dma_start(x_ew[:], xew_in[b])
        state["x_ew", b] = x_ew

    def phase_B_bundles(b):
        """elementwise tail + life masks + store for batch b (thunk list)."""
        x_ew = state.pop(("x_ew", b))
        st = {}

        def bcast(t128):
            return bass.AP(tensor=t128.tensor, offset=t128.offset,
                           ap=[[t128.ap[0][0], H], [0, C], [1, W]])

        def bn_ew():
            dx_ew = state.pop(("dx_ew", b))
            m_b = bass.AP(tensor=m_all.tensor, offset=m_all.offset + b * W,
                          ap=[[m_all.ap[0][0], H], [0, C], [1, W]])
            nc.vector.tensor_tensor(dx_ew[:], dx_ew[:], m_b,
                                    op=mybir.AluOpType.mult)
            xnew = ewpool.tile([H, CW], F32, name=f"xnew{b}", tag="xnew")
            nc.vector.tensor_tensor(xnew[:], x_ew[:], dx_ew[:],
                                    op=mybir.AluOpType.add)
            st["xnew"] = xnew

        def living(src_getter, which):
            def fn():
                src_ew = src_getter()
                ap_pad = small.tile([H, SW], F32, name=f"ap{which}{b}",
                                    tag=f"ap{which}")
                alpha = src_ew[:, 3 * W:4 * W]
                nc.vector.tensor_copy(ap_pad[:, 1:1 + W], alpha)
                nc.vector.tensor_copy(ap_pad[:, 0:1],
                                      src_ew[:, 4 * W - 1:4 * W])
                nc.vector.tensor_copy(ap_pad[:, 1 + W:2 + W],
                                      src_ew[:, 3 * W:3 * W + 1])
                hh = small.tile([H, 2 * W], F32, name=f"hh{which}{b}",
                                tag=f"hh{which}")
                hm = hh[:, 0:W]
                hs = hh[:, W:2 * W]
                nc.vector.tensor_tensor(hm, ap_pad[:, 0:W], ap_pad[:, 1:1 + W],
                                        op=mybir.AluOpType.max)
                nc.vector.tensor_tensor(hm, hm, ap_pad[:, 2:2 + W],
                                        op=mybir.AluOpType.max)
                nc.vector.tensor_tensor(hs, ap_pad[:, 0:W], ap_pad[:, 1:1 + W],
                                        op=mybir.AluOpType.add)
                nc.vector.tensor_tensor(hs, hs, ap_pad[:, 2:2 + W],
                                        op=mybir.AluOpType.add)
                up = small.tile([H, 2 * W], F32, name=f"up{which}{b}",
                                tag=f"up{which}")
                dn = small.tile([H, 2 * W], F32, name=f"dn{which}{b}",
                                tag=f"dn{which}")
                nc.gpsimd.dma_start(up[0:H - 1, :], hh[1:H, :])
                nc.gpsimd.dma_start(up[H - 1:H, :], hh[0:1, :])
                nc.gpsimd.dma_start(dn[1:H, :], hh[0:H - 1, :])
                nc.gpsimd.dma_start(dn[0:1, :], hh[H - 1:H, :])
                st[f"hh{which}"] = (hh, up, dn)
            return fn

        def living_v(which):
            def fn():
                hh, up, dn = st.pop(f"hh{which}")
                vm = small.tile([H, W], F32, name=f"vm{which}{b}",
                                tag=f"vm{which}")
                vs = small.tile([H, W], F32, name=f"vs{which}{b}",
                                tag=f"vs{which}")
                for (t_out, o0, op) in ((vm, 0, mybir.AluOpType.max),
                                        (vs, W, mybir.AluOpType.add)):
                    nc.vector.tensor_tensor(t_out[:], hh[:, o0:o0 + W],
                                            up[:, o0:o0 + W], op=op)
                    nc.vector.tensor_tensor(t_out[:], t_out[:],
                                            dn[:, o0:o0 + W], op=op)
                alive = small.tile([H, W], F32, name=f"al{which}{b}",
                                   tag=f"al{which}")
                nc.vector.tensor_scalar(alive[:], vm[:], 0.1, None,
                                        op0=mybir.AluOpType.is_gt)
                avgok = small.tile([H, W], F32, name=f"ag{which}{b}",
                                   tag=f"ag{which}")
                nc.vector.tensor_scalar(avgok[:], vs[:], AVG_LT, None,
                                        op0=mybir.AluOpType.is_lt)
                lif = small.tile([H, W], F32, name=f"lf{which}{b}",
                                 tag=f"lf{which}")
                nc.vector.tensor_tensor(lif[:], alive[:], avgok[:],
                                        op=mybir.AluOpType.mult)
                st[f"life{which}"] = lif
            return fn

        def bn_final():
            xnew = st["xnew"]
            life = small.tile([H, W], F32, name=f"life{b}", tag="life")
            nc.vector.tensor_tensor(life[:], st["lifepre"][:],
                                    st["lifepost"][:],
                                    op=mybir.AluOpType.mult)
            nc.vector.tensor_tensor(xnew[:], xnew[:], bcast(life),
                                    op=mybir.AluOpType.mult)
            nc.sync.dma_start(out_dram[b], xnew[:])

        pre = [
            living(lambda: x_ew, "pre"),
            living_v("pre"),
        ]
        post = [
            bn_ew,
            living(lambda: st["xnew"], "post"),
            living_v("post"),
            bn_final,
        ]
        return pre, post

    # --- pipeline: (b, q) steps, dripping phase-B thunks between MM blocks:
    # pre-life of batch b runs during b's own quarters; the tail (xnew,
    # post-life, store) runs during b+1's quarters.
    pending = []

    def drip():
        if pending:
            pending.pop(0)()

    NSTEP = B_LOC * NQ
    PREFETCH = 4                  # octants of lookahead (= 2 load-quarters)
    bundles = {}
    load_stacks(0, 0)
    load_consts()
    load_stacks(0, 1)
    prev = None
    for step in range(NSTEP):
        b, q = divmod(step, NQ)
        if step == 1:
            # deferred past the startup burst: only needed by phase B
            load_xew(0)
            nc.gpsimd.dma_start(m_all[:], m_in[:])
        if q == 1:
            pre, post = phase_B_bundles(b)
            bundles[b] = post
            pending.extend(pre)
        h_sb = mlp1_part(b, q, drip)
        if prev is not None:
            pb, pq, ph = prev
            mlp2_part(pb, pq, ph, drip)
            if pq == NQ - 1:
                transpose_dx(pb)
                while pending:
                    drip()
                pending = bundles.pop(pb)
        prev = (b, q, h_sb)
        # loads issued after the dumps so a buffer-gated load never
        # head-blocks them on the sync queue
        if step + PREFETCH < NSTEP and (step + PREFETCH) % 2 == 0:
            nb, nq = divmod(step + PREFETCH, NQ)
            load_stacks(nb, nq // 2)
            if nq == 0:
                load_xew(nb)
    pb, pq, ph = prev
    mlp2_part(pb, pq, ph, drip)
    transpose_dx(pb)
    while pending:
        drip()
    pending = bundles.pop(pb)
    while pending:
        drip()


# ----------------------------------------------------------------------------
_PROGRAM_CACHE = {}


def _get_program():
    key = (LRELU_MODE,)
    if key in _PROGRAM_CACHE:
        return _PROGRAM_CACHE[key]
    nc = bacc.Bacc("TRN2", target_bir_lowering=False, debug=False,
                   num_devices=N_CORES)
    xrep1_in = nc.dram_tensor("xrep1_in", [B_LOC, 128, RT], BF16,
                              kind="ExternalInput").ap()
    xrep2_in = nc.dram_tensor("xrep2_in", [B_LOC, 128, RT], BF16,
                              kind="ExternalInput").ap()
    xrep3_in = nc.dram_tensor("xrep3_in", [B_LOC, 49, RT], BF16,
                              kind="ExternalInput").ap()
    xew_in = nc.dram_tensor("xew_in", [B_LOC, H, CW], F32,
                            kind="ExternalInput").ap()
    m_in = nc.dram_tensor("m_in", [H, B_LOC * W], F32,
                          kind="ExternalInput").ap()
    wa1_in = nc.dram_tensor("wa1_in", [128, HID], BF16,
                            kind="ExternalInput").ap()
    wa2_in = nc.dram_tensor("wa2_in", [128, HID], BF16,
                            kind="ExternalInput").ap()
    wa4_in = nc.dram_tensor("wa4_in", [113, HID], BF16,
                            kind="ExternalInput").ap()
    w2_in = nc.dram_tensor("w2_in", [HID, 32], F32, kind="ExternalInput").ap()
    b2_in = nc.dram_tensor("b2_in", [HID, 1], F32, kind="ExternalInput").ap()
    out_dram = nc.dram_tensor("out", [B_LOC, H, CW], F32,
                              kind="ExternalOutput").ap()
    scr_drams = [nc.dram_tensor(f"dxscr{b}", [H, C, W], F32).ap()
                 for b in range(B_LOC)]
    with tile.TileContext(nc) as tc:
        _build_kernel(tc, xrep1_in, xrep2_in, xrep3_in, xew_in, m_in,
                      wa1_in, wa2_in, wa4_in, w2_in, b2_in, out_dram,
                      scr_drams)
    nc.compile()
    _PROGRAM_CACHE[key] = nc
    return nc


def _host_weights(filters, W1, b1, W2, b2):
    import ml_dtypes
    filters = np.asarray(filters, np.float32)
    W1 = np.asarray(W1, np.float32)
    W2 = np.asarray(W2, np.float32)
    b1 = np.asarray(b1, np.float32)
    b2 = np.asarray(b2, np.float32)
    # Weff[o, c, d, dj] = sum_f W1[o, c*NF+f] * filters[f, d, dj]
    w1r = W1.reshape(HID, C, NF)
    weff = np.einsum("ocf,fij->ocij", w1r, filters)       # [o, c, d, dj]
    wah = weff.astype(ml_dtypes.bfloat16)
    wal = (weff - wah.astype(np.float32)).astype(ml_dtypes.bfloat16)
    wa1 = np.zeros((128, HID), ml_dtypes.bfloat16)
    wa2 = np.zeros((128, HID), ml_dtypes.bfloat16)
    for k, (dj, d) in enumerate(COMBOS[:8]):
        wa1[k * C:(k + 1) * C, :] = wah[:, :, d, dj].T
        wa2[k * C:(k + 1) * C, :] = wal[:, :, d, dj].T
    wa4 = np.zeros((113, HID), ml_dtypes.bfloat16)
    wa4[0:C, :] = wah[:, :, 2, 2].T
    wa4[C:2 * C, :] = wal[:, :, 2, 2].T
    wa4[2 * C:3 * C, :] = wah[:, :, 2, 2].T
    wa4[48, :] = b1.astype(ml_dtypes.bfloat16)
    wa4[64:113, :] = wa4[0:49, :]
    w2p = np.zeros((HID, 32), np.float32)
    w2p[:, :C] = W2.T
    b2v = np.zeros((HID, 1), np.float32)
    for j in range(4):
        b2v[32 * j:32 * j + C, 0] = b2
    return wa1, wa2, wa4, w2p, b2v


def _prepare_in_maps(x, rand_mask, filters, W1, b1, W2, b2):
    import ml_dtypes
    x = np.asarray(x, np.float32)
    # replica planes: xp rows/cols -1..128 (wrap); combo (dj,d) plane =
    # xp[:, :, d:d+128, dj:dj+128] == x[c, i+d-1, j+dj-1]
    xp = np.pad(x, ((0, 0), (0, 0), (1, 1), (1, 1)), mode="wrap")
    xp_h = xp.astype(ml_dtypes.bfloat16)
    xp_l = (xp - xp_h.astype(np.float32)).astype(ml_dtypes.bfloat16)
    xrep1 = np.empty((B, 8, C, H, W), ml_dtypes.bfloat16)
    xrep2 = np.empty((B, 8, C, H, W), ml_dtypes.bfloat16)
    for k, (dj, d) in enumerate(COMBOS[:8]):
        xrep1[:, k] = xp_h[:, :, d:d + H, dj:dj + W]
        xrep2[:, k] = xp_l[:, :, d:d + H, dj:dj + W]
    xrep1 = xrep1.reshape(B, 128, RT)
    xrep2 = xrep2.reshape(B, 128, RT)
    xrep3 = np.empty((B, 49, H, W), ml_dtypes.bfloat16)
    xrep3[:, 0:C] = xp_h[:, :, 2:2 + H, 2:2 + W]
    xrep3[:, C:2 * C] = xrep3[:, 0:C]
    xrep3[:, 2 * C:3 * C] = xp_l[:, :, 2:2 + H, 2:2 + W]
    xrep3[:, 48] = np.ones((B, H, W), ml_dtypes.bfloat16)
    xrep3 = xrep3.reshape(B, 49, RT)
    xew = np.ascontiguousarray(x.transpose(0, 2, 1, 3).reshape(B, H, CW))
    m = (np.asarray(rand_mask, np.float32)
         <= np.float32(FIRE_RATE)).astype(np.float32)
    m = m.reshape(B, H, W).transpose(1, 0, 2)   # [H, B, W]
    wa1, wa2, wa4, w2p, b2v = _host_weights(filters, W1, b1, W2, b2)
    in_maps = []
    for core in range(N_CORES):
        sl = slice(core * B_LOC, (core + 1) * B_LOC)
        in_maps.append({
            "xrep1_in": xrep1[sl], "xrep2_in": xrep2[sl],
            "xrep3_in": xrep3[sl], "xew_in": xew[sl],
            "m_in": np.ascontiguousarray(m[:, sl, :]).reshape(H, B_LOC * W),
            "wa1_in": wa1, "wa2_in": wa2, "wa4_in": wa4,
            "w2_in": w2p, "b2_in": b2v,
        })
    return in_maps


def kernel(x, rand_mask, filters, W1, b1, W2, b2, _want_trace=False):
    in_maps = _prepare_in_maps(x, rand_mask, filters, W1, b1, W2, b2)
    nc = _get_program()
    res = run_bass_kernel_spmd(nc, in_maps, list(range(N_CORES)),
                               trace=_want_trace)
    out = np.concatenate([res.results[i]["out"] for i in range(N_CORES)],
                         axis=0)
    out = np.ascontiguousarray(
        out.reshape(B, H, C, W).transpose(0, 2, 1, 3))
    if _want_trace:
        return out, res
    return out
